# revision 1
# baseline (speedup 1.0000x reference)
"""Llama GQA attention (B=2, S=2048, H=4096, 32 q heads / 8 kv heads, HD=128)
on 8 Trainium2 NeuronCores.

Sharding: DP=2 over batch x TP=4 over heads.
  core c: batch b = c // 4, tp rank r = c % 4
  - owns q heads [8r, 8r+8), kv heads [2r, 2r+2)
  - computes attention for its heads over its batch
  - AllGather (bf16) of attention outputs within each 4-core batch group
  - output projection over the full 4096 attn features for output columns
    [1024r, 1024(r+1)) -> disjoint outputs, host concatenates.

On-chip layout is fully "transposed" ([feature, token]) so no transposes are
needed anywhere:
  QT/KT: [d, t] (weight tiles stationary, X^T moving)
  V:     [t, d] (X^T tiles stationary, wv moving)
  S^T[k, q] = (KT tile).T @ QT          (contraction d on partitions)
  P^T = exp(scale * S^T)                (ScalarE, fp32 PSUM -> bf16 SBUF)
  attn^T[d, q] += (V tile).T @ P^T      (contraction k-tokens on partitions)
  denom[*, q] += ones128.T @ P^T        (col-sums replicated on all partitions)
  out^T[oc, t] += (wo tile).T @ attn_full^T
Causal masking: only lower-triangle k-tiles are computed; the 4 diagonal
128x512 tiles per q-block use one of 4 static 0/1 masks (multiplied into P^T).
Softmax skips max-subtraction (scores are O(7), exp fits fp32 comfortably).

Engine-stream pipelining (per-engine programs execute in order, so Python
emission order IS the schedule skeleton):
  iter c:  proj(c) -> outproj(c-1) -> attention(c) [with xt(c+1) casts
           interleaved on DVE] -> AllGather(c)
GpSimd hosts ONLY the chunk-0 weight-panel casts and the collectives, so a
collective's completion wait never blocks next-chunk work.
"""

import os
import sys

for _p in ("/opt/trn_rl_repo",):
    if _p not in sys.path:
        sys.path.append(_p)

import math
import numpy as np
import ml_dtypes

import concourse.bacc as bacc
import concourse.mybir as mybir
import concourse.tile as tile
from concourse.bass_utils import run_bass_kernel_spmd

F32 = mybir.dt.float32
BF16 = mybir.dt.bfloat16

B, S, H = 2, 2048, 4096
NH, NKV, HD = 32, 8, 128
N_CORES = 8
TP = 4
GROUPS = [[0, 1, 2, 3], [4, 5, 6, 7]]

HL = NH // TP          # 8 local q heads
KVL = NKV // TP        # 2 local kv heads
QCOLS = HL * HD        # 1024 local q cols
KVCOLS = KVL * HD      # 256 local kv cols
OC = H // TP           # 1024 local out cols

TC = 512               # token chunk (= one attention q-block)
NCHUNK = S // TC       # 4
KT = H // 128          # 32 contraction tiles for the projections
DT = H // 128          # 32 contraction tiles for the out projection
SCALE = float(HD ** -0.5)

# weight panels, all [128 k-part, 32kt * 128 c] (free = kt*128 + c):
#   0..7   wq head panels (lhsT)
#   8..9   wk kv-head panels (lhsT)
#   10..11 wv half-panels (rhs, 128 cols each)
#   12..19 wo out-col panels (lhsT)
NPANEL = 20

LAST_RESULT = None
_BUILT = {}


def _build():
    nc = bacc.Bacc("TRN2", debug=False, num_devices=N_CORES)

    xt_d = nc.dram_tensor("xt", [H, S], F32, kind="ExternalInput").ap()
    cos_d = nc.dram_tensor("cos_t", [HD, S], F32, kind="ExternalInput").ap()
    sin_d = nc.dram_tensor("sin_t", [HD, S], F32, kind="ExternalInput").ap()
    wq_d = nc.dram_tensor("wq_s", [H, QCOLS], F32, kind="ExternalInput").ap()
    wk_d = nc.dram_tensor("wk_s", [H, KVCOLS], F32, kind="ExternalInput").ap()
    wv_d = nc.dram_tensor("wv_s", [H, KVCOLS], F32, kind="ExternalInput").ap()
    wo_d = nc.dram_tensor("wo_s", [H, OC], F32, kind="ExternalInput").ap()
    mask_d = nc.dram_tensor("masks", [HD, 4 * TC], BF16, kind="ExternalInput").ap()
    ones_d = nc.dram_tensor("onesb", [128, 128], BF16, kind="ExternalInput").ap()
    out_d = nc.dram_tensor("out_t", [OC, S], F32, kind="ExternalOutput").ap()

    with tile.TileContext(nc) as tc:
        with tc.tile_pool(name="sb", bufs=1) as sb, \
             tc.tile_pool(name="ps", bufs=1, space="PSUM") as ps, \
             tc.tile_pool(name="dr", bufs=1, space="DRAM") as dr:

            # ---- persistent tiles ----
            cos_sb = sb.tile([HD, S], F32)
            sin_sb = sb.tile([HD, S], F32)
            mask_sb = sb.tile([HD, 4 * TC], BF16)
            ones_sb = sb.tile([128, 128], BF16)
            ktb = sb.tile([128, KVL * S], BF16)      # roped K^T, [d, kv*S + t]
            vb = sb.tile([128, (S // 128) * KVCOLS], BF16)  # V, [t, tt*256 + d]
            nc.sync.dma_start(cos_sb[:], cos_d[:])
            nc.sync.dma_start(sin_sb[:], sin_d[:])
            nc.sync.dma_start(mask_sb[:], mask_d[:])
            nc.sync.dma_start(ones_sb[:], ones_d[:])

            wscr = dr.tile([NPANEL, 128, KT * 128], BF16)

            def panel_src(idx):
                """fp32 dram AP for panel idx, as [p, kt, c] (p = k % 128)."""
                if idx < 8:
                    return wq_d[:, idx * 128:(idx + 1) * 128]
                if idx < 10:
                    return wk_d[:, (idx - 8) * 128:(idx - 7) * 128]
                if idx < 12:
                    return wv_d[:, (idx - 10) * 128:(idx - 9) * 128]
                return wo_d[:, (idx - 12) * 128:(idx - 11) * 128]

            def get_panel(c, idx):
                """[128, KT*128] bf16 panel; cast+stash to DRAM scratch on
                chunk 0 (cast on GpSimd), read back from scratch afterwards."""
                wb = sb.tile([128, KT * 128], BF16, tag="wb", bufs=2)
                if c == 0:
                    wst = sb.tile([128, KT * 128], F32, tag="wst", bufs=2)
                    src = panel_src(idx).rearrange("(kt p) c -> p kt c", p=128)
                    nc.sync.dma_start(wst.rearrange("p (kt c) -> p kt c", c=128), src)
                    # split the cast between DVE and ACT (GpSimd is ~4x
                    # slower per element and would serialize chunk 0)
                    half = KT * 64
                    nc.vector.tensor_copy(wb[:, :half], wst[:, :half])
                    nc.scalar.copy(wb[:, half:], wst[:, half:])
                    nc.sync.dma_start(wscr[idx], wb[:])
                else:
                    nc.sync.dma_start(wb[:], wscr[idx])
                return wb

            def load_xt_tile(c, kt):
                """DMA one [128, TC] fp32 tile of X^T and cast to bf16 (DVE)."""
                t0 = c * TC
                xst = sb.tile([128, TC], F32, tag="xst", bufs=3)
                nc.sync.dma_start(
                    xst[:], xt_d[kt * 128:(kt + 1) * 128, t0:t0 + TC])
                dst = _XTB[c][:, kt * TC:(kt + 1) * TC]
                if kt % 2 == 0:
                    nc.vector.tensor_copy(dst, xst[:])
                else:
                    nc.scalar.copy(dst, xst[:])

            def rope(dst, pq, t0):
                """dst (bf16 [128, TC]) = rope of pq (fp32 PSUM [128, TC])."""
                qf = sb.tile([128, TC], F32, tag="qf", bufs=2)
                nc.scalar.copy(qf[:], pq[:])
                qs = sb.tile([128, TC], F32, tag="qs", bufs=2)
                nc.sync.dma_start(qs[0:64, :], qf[64:128, :])
                nc.sync.dma_start(qs[64:128, :], qf[0:64, :])
                nc.vector.tensor_tensor(
                    qf[:], qf[:], cos_sb[:, t0:t0 + TC], mybir.AluOpType.mult)
                nc.vector.tensor_tensor(
                    qs[:], qs[:], sin_sb[:, t0:t0 + TC], mybir.AluOpType.mult)
                nc.vector.tensor_tensor(dst, qf[:], qs[:], mybir.AluOpType.add)

            _XTB = {}

            def proj(c):
                t0 = c * TC
                qtb = sb.tile([128, HL * TC], BF16, tag="qt", bufs=2, name="qtb")
                for h in range(HL):
                    wb = get_panel(c, h)
                    pq = ps.tile([128, TC], F32, tag="pj", bufs=2, name="pq")
                    for kt in range(KT):
                        nc.tensor.matmul(
                            pq[:], wb[:, kt * 128:(kt + 1) * 128],
                            _XTB[c][:, kt * TC:(kt + 1) * TC],
                            start=(kt == 0), stop=(kt == KT - 1))
                    rope(qtb[:, h * TC:(h + 1) * TC], pq, t0)
                for kv in range(KVL):
                    wb = get_panel(c, 8 + kv)
                    pk = ps.tile([128, TC], F32, tag="pj", bufs=2, name="pk")
                    for kt in range(KT):
                        nc.tensor.matmul(
                            pk[:], wb[:, kt * 128:(kt + 1) * 128],
                            _XTB[c][:, kt * TC:(kt + 1) * TC],
                            start=(kt == 0), stop=(kt == KT - 1))
                    rope(ktb[:, kv * S + t0:kv * S + t0 + TC], pk, t0)
                wv0 = get_panel(c, 10)
                wv1 = get_panel(c, 11)
                for tt_ in range(TC // 128):
                    # separate psum banks per half: start=True clears
                    # has_written for the WHOLE bank.
                    pv0 = ps.tile([128, 128], F32, tag="pj", bufs=2, name="pv0")
                    pv1 = ps.tile([128, 128], F32, tag="pj", bufs=2, name="pv1")
                    for kt in range(KT):
                        lx = _XTB[c][:, kt * TC + tt_ * 128:kt * TC + (tt_ + 1) * 128]
                        nc.tensor.matmul(
                            pv0[:], lx, wv0[:, kt * 128:(kt + 1) * 128],
                            start=(kt == 0), stop=(kt == KT - 1))
                        nc.tensor.matmul(
                            pv1[:], lx, wv1[:, kt * 128:(kt + 1) * 128],
                            start=(kt == 0), stop=(kt == KT - 1))
                    vt_idx = (t0 // 128) + tt_
                    nc.scalar.copy(
                        vb[:, vt_idx * KVCOLS:vt_idx * KVCOLS + 128], pv0[:])
                    nc.scalar.copy(
                        vb[:, vt_idx * KVCOLS + 128:(vt_idx + 1) * KVCOLS], pv1[:])
                return qtb

            def attention(c, qtb):
                """Attention for q block c; interleaves xt(c+1) loads on DVE."""
                nkt = 4 * c + 4
                attnb = sb.tile([128, HL * TC], BF16, tag="attn", bufs=2,
                                name="attnb")
                prefetch = list(range(KT)) if c + 1 < NCHUNK else []
                for h in range(HL):
                    kv = h // (HL // KVL)
                    qsl = qtb[:, h * TC:(h + 1) * TC]
                    pa = ps.tile([128, TC], F32, tag="pa", bufs=2, name="pa")
                    pd = ps.tile([128, TC], F32, tag="den", bufs=1, name="pd")
                    for kt in range(nkt):
                        sps = ps.tile([128, TC], F32, tag="s", bufs=2, name="sps")
                        nc.tensor.matmul(
                            sps[:],
                            ktb[:, kv * S + kt * 128:kv * S + (kt + 1) * 128],
                            qsl, start=True, stop=True)
                        pt = sb.tile([128, TC], BF16, tag="pt", bufs=3, name="pt")
                        nc.scalar.activation(
                            pt[:], sps[:], mybir.ActivationFunctionType.Exp,
                            scale=SCALE)
                        j = kt - 4 * c
                        if j >= 0:
                            # on GpSimd: keeps the (busy) DVE out of the
                            # exp->mask->matmul critical chain
                            nc.gpsimd.tensor_tensor(
                                pt[:], pt[:], mask_sb[:, j * TC:(j + 1) * TC],
                                mybir.AluOpType.mult)
                        nc.tensor.matmul(
                            pa[:],
                            vb[:, kt * KVCOLS + kv * 128:kt * KVCOLS + (kv + 1) * 128],
                            pt[:], start=(kt == 0), stop=(kt == nkt - 1))
                        nc.tensor.matmul(
                            pd[:], ones_sb[:], pt[:],
                            start=(kt == 0), stop=(kt == nkt - 1))
                    rc = sb.tile([128, TC], F32, tag="rc", bufs=2, name="rc")
                    nc.vector.reciprocal(rc[:], pd[:])
                    nc.vector.tensor_tensor(
                        attnb[:, h * TC:(h + 1) * TC], pa[:], rc[:],
                        mybir.AluOpType.mult)
                    # interleave next-chunk X^T loads between heads (keeps
                    # DVE mask/normalize latency low while still
                    # prefetching)
                    for _ in range(4):
                        if prefetch:
                            load_xt_tile(c + 1, prefetch.pop(0))
                while prefetch:
                    load_xt_tile(c + 1, prefetch.pop(0))
                return attnb

            def allgather(c, attnb):
                cci = dr.tile([QCOLS, TC], BF16, tag="cci", bufs=2, name="cci")
                cco = dr.tile([H, TC], BF16, tag="cco", bufs=2, name="cco")
                nc.sync.dma_start(
                    cci.rearrange("(h p) t -> p h t", p=128),
                    attnb.rearrange("p (h t) -> p h t", h=HL))
                nc.gpsimd.collective_compute(
                    "AllGather", mybir.AluOpType.bypass,
                    replica_groups=GROUPS, ins=[cci[:]], outs=[cco[:]])
                return cco

            def outproj(c, cco):
                t0 = c * TC
                af = sb.tile([128, DT * TC], BF16, tag="big", bufs=2, name="af")
                nc.sync.dma_start(
                    af.rearrange("p (d t) -> p d t", d=DT),
                    cco.rearrange("(d p) t -> p d t", p=128))
                for o in range(OC // 128):
                    wb = get_panel(c, 12 + o)
                    po = ps.tile([128, TC], F32, tag="op", bufs=1, name="po")
                    for dt_ in range(DT):
                        nc.tensor.matmul(
                            po[:], wb[:, dt_ * 128:(dt_ + 1) * 128],
                            af[:, dt_ * TC:(dt_ + 1) * TC],
                            start=(dt_ == 0), stop=(dt_ == DT - 1))
                    ot = sb.tile([128, TC], F32, tag="ot", bufs=2, name="ot")
                    nc.scalar.copy(ot[:], po[:])
                    nc.sync.dma_start(
                        out_d[o * 128:(o + 1) * 128, t0:t0 + TC], ot[:])

            # ---- software-pipelined chunk loop ----
            _XTB[0] = sb.tile([128, KT * TC], BF16, tag="big", bufs=2,
                              name="xtb0")
            for kt in range(KT):
                load_xt_tile(0, kt)

            # slot rotation for tag "big" (bufs=2): xtb0 A, xtb1 B, af0 A,
            # xtb2 B, af1 A, xtb3 B, af2 A, af3 B — each af(c-1) takes the
            # slot xtb(c-1) freed after proj(c-1), so af DMA overlaps proj(c)
            # and xt prefetch overlaps attention.
            pending = None   # (chunk, cco) awaiting out projection
            for c in range(NCHUNK):
                qtb = proj(c)
                if pending is not None:
                    outproj(*pending)
                if c + 1 < NCHUNK:
                    _XTB[c + 1] = sb.tile([128, KT * TC], BF16, tag="big",
                                          bufs=2, name=f"xtb{c + 1}")
                attnb = attention(c, qtb)
                cco = allgather(c, attnb)
                pending = (c, cco)
            outproj(*pending)

    nc.compile()
    return nc


def _get_nc():
    if "nc" not in _BUILT:
        _BUILT["nc"] = _build()
    return _BUILT["nc"]


def kernel(hidden_states, cos, sin, wq, wk, wv, wo):
    global LAST_RESULT
    nc = _get_nc()

    hidden_states = np.asarray(hidden_states, dtype=np.float32)
    cos = np.asarray(cos, dtype=np.float32)
    sin = np.asarray(sin, dtype=np.float32)
    wq = np.asarray(wq, dtype=np.float32)
    wk = np.asarray(wk, dtype=np.float32)
    wv = np.asarray(wv, dtype=np.float32)
    wo = np.asarray(wo, dtype=np.float32)

    # host-side shard prep
    xts = [np.ascontiguousarray(hidden_states[b].T) for b in range(B)]
    cts = [np.ascontiguousarray(cos[b].T) for b in range(B)]
    sin_eff = []
    for b in range(B):
        st = np.ascontiguousarray(sin[b].T)
        se = st.copy()
        se[0:64, :] *= -1.0
        sin_eff.append(se)

    masks = np.zeros((HD, 4 * TC), dtype=ml_dtypes.bfloat16)
    for j in range(4):
        m = (np.arange(HD)[:, None] + 128 * j) <= np.arange(TC)[None, :]
        masks[:, j * TC:(j + 1) * TC] = m.astype(ml_dtypes.bfloat16)
    onesb = np.ones((128, 128), dtype=ml_dtypes.bfloat16)

    in_maps = []
    for core in range(N_CORES):
        b, r = core // TP, core % TP
        in_maps.append({
            "xt": xts[b],
            "cos_t": cts[b],
            "sin_t": sin_eff[b],
            "wq_s": np.ascontiguousarray(wq[:, r * QCOLS:(r + 1) * QCOLS]),
            "wk_s": np.ascontiguousarray(wk[:, r * KVCOLS:(r + 1) * KVCOLS]),
            "wv_s": np.ascontiguousarray(wv[:, r * KVCOLS:(r + 1) * KVCOLS]),
            "wo_s": np.ascontiguousarray(wo[:, r * OC:(r + 1) * OC]),
            "masks": masks,
            "onesb": onesb,
        })

    res = run_bass_kernel_spmd(nc, in_maps, core_ids=list(range(N_CORES)))
    LAST_RESULT = res

    out = np.empty((B, S, H), dtype=np.float32)
    for core in range(N_CORES):
        b, r = core // TP, core % TP
        out[b, :, r * OC:(r + 1) * OC] = res.results[core]["out_t"].T
    return out



# revision 3
# speedup vs baseline: 1.0271x; 1.0271x over previous
"""Llama GQA attention (B=2, S=2048, H=4096, 32 q heads / 8 kv heads, HD=128)
on 8 Trainium2 NeuronCores.

Sharding: DP=2 over batch x TP=4 over heads.
  core c: batch b = c // 4, tp rank r = c % 4
  - owns q heads [8r, 8r+8), kv heads [2r, 2r+2)
  - computes attention for its heads over its batch
  - o_proj: LOCAL partial over its 1024 attn features for ALL 4096 out
    cols (wo sharded by ROWS), then bf16 ReduceScatter(add) within each
    4-core batch group -> each core holds out cols [1024r, 1024(r+1)).

All inputs are pre-cast to bf16 on the host (weights in panel-major
layouts so every DMA is contiguous), so the device pipeline has zero
dtype-conversion work and no DRAM scratch round-trips.

On-chip layout is fully "transposed" ([feature, token]):
  QT/KT: [d, t] (weight tiles stationary, X^T moving)
  V:     [t, d] (X^T tiles stationary, wv moving)
  S^T[k, q] = (KT tile).T @ QT          (contraction d on partitions)
  P^T = exp(scale * S^T)                (ScalarE, fp32 PSUM -> bf16 SBUF)
  attn^T[d, q] += (V tile).T @ P^T      (contraction k-tokens on partitions)
  out[t, oc]  += (attnb tile).T @ wo    (contraction d on partitions)
Causal masking: only lower-triangle k-tiles are computed; diagonal
128x512 tiles use one of 4 static 0/1 masks (multiplied into P^T on DVE).
Softmax skips max-subtraction (scores are O(7), exp fits fp32).

Denominators: P^T tiles are accumulated on the Pool engine into a
per-head running sum (bf16), then ONE ones-matmul per (head, chunk)
produces the denominator — instead of one matmul per k-tile.

Attention processes heads in PAIRS sharing the same kv head, software-
pipelined so the exp->mask->pa chain of one head hides under the other
head's matmuls:
  per kt: [score(h0), score(h1), pa(h0, kt-1), pa(h1, kt-1)]
X^T for chunk c+1 is DMA'd (pure bf16 copy, no engine work) into a
2-slot rotation at the start of proj(c).
"""

import os
import sys

for _p in ("/opt/trn_rl_repo",):
    if _p not in sys.path:
        sys.path.append(_p)

import numpy as np
import ml_dtypes

import concourse.bacc as bacc
import concourse.mybir as mybir
import concourse.tile as tile
from concourse.bass_utils import run_bass_kernel_spmd

F32 = mybir.dt.float32
BF16 = mybir.dt.bfloat16
B16NP = ml_dtypes.bfloat16

B, S, H = 2, 2048, 4096
NH, NKV, HD = 32, 8, 128
N_CORES = 8
TP = 4
GROUPS = [[0, 1, 2, 3], [4, 5, 6, 7]]

HL = NH // TP          # 8 local q heads
KVL = NKV // TP        # 2 local kv heads
QCOLS = HL * HD        # 1024 local q cols
KVCOLS = KVL * HD      # 256 local kv cols
OC = H // TP           # 1024 out cols per core after ReduceScatter

TC = 512               # token chunk (= one attention q-block)
NCHUNK = S // TC       # 4
KT = H // 128          # 32 contraction tiles for the projections
SCALE = float(HD ** -0.5)

LAST_RESULT = None
_BUILT = {}

EXP = mybir.ActivationFunctionType.Exp
MUL = mybir.AluOpType.mult
ADD = mybir.AluOpType.add


def _build():
    nc = bacc.Bacc("TRN2", debug=False, num_devices=N_CORES)

    xt_d = nc.dram_tensor("xt", [H, S], BF16, kind="ExternalInput").ap()
    cos_d = nc.dram_tensor("cos_t", [HD, S], BF16, kind="ExternalInput").ap()
    sin_d = nc.dram_tensor("sin_t", [HD, S], BF16, kind="ExternalInput").ap()
    wq_d = nc.dram_tensor("wqp", [HL, 128, KT * 128], BF16, kind="ExternalInput").ap()
    wk_d = nc.dram_tensor("wkp", [KVL, 128, KT * 128], BF16, kind="ExternalInput").ap()
    wv_d = nc.dram_tensor("wvp", [128, KT * KVCOLS], BF16, kind="ExternalInput").ap()
    wo_d = nc.dram_tensor("wop", [HL, 128, H], BF16, kind="ExternalInput").ap()
    mask_d = nc.dram_tensor("masks", [HD, 4 * TC], BF16, kind="ExternalInput").ap()
    ones_d = nc.dram_tensor("onesb", [128, 128], BF16, kind="ExternalInput").ap()
    out_d = nc.dram_tensor("out_t", [S, OC], BF16, kind="ExternalOutput").ap()

    with tile.TileContext(nc) as tc:
        with tc.tile_pool(name="sb", bufs=1) as sb, \
             tc.tile_pool(name="ps", bufs=1, space="PSUM") as ps, \
             tc.tile_pool(name="dr", bufs=1, space="DRAM") as dr:

            # ---- persistent tiles ----
            cos_sb = sb.tile([HD, S], BF16)
            sin_sb = sb.tile([HD, S], BF16)
            mask_sb = sb.tile([HD, 4 * TC], BF16)
            ones_sb = sb.tile([128, 128], BF16)
            ktb = sb.tile([128, KVL * S], BF16)             # roped K^T
            vb = sb.tile([128, (S // 128) * KVCOLS], BF16)  # V, [t, tt*256 + d]
            # X^T bf16, 2-chunk rotation: [128, kt*(2*TC) + slot*TC + t]
            xtb = sb.tile([128, KT * 2 * TC], BF16)
            nc.sync.dma_start(cos_sb[:], cos_d[:])
            nc.sync.dma_start(sin_sb[:], sin_d[:])
            nc.sync.dma_start(mask_sb[:], mask_d[:])
            nc.sync.dma_start(ones_sb[:], ones_d[:])

            def xt_slot(c):
                return c % 2

            def load_xt(c):
                """Issue the 32 tile DMAs bringing chunk c of X^T into its
                rotation slot (pure bf16 copy, no engine work)."""
                sl = xt_slot(c)
                for kt in range(KT):
                    dst = xtb[:, kt * 2 * TC + sl * TC:kt * 2 * TC + (sl + 1) * TC]
                    nc.sync.dma_start(
                        dst, xt_d[kt * 128:(kt + 1) * 128, c * TC:(c + 1) * TC])

            def xt_tile(c, kt, lo=0, width=TC):
                sl = xt_slot(c)
                base = kt * 2 * TC + sl * TC + lo
                return xtb[:, base:base + width]

            def rope(dst, pq, t0):
                """dst (bf16 [128, TC]) = rope of pq (fp32 PSUM [128, TC])."""
                qf = sb.tile([128, TC], F32, tag="qf", bufs=2)
                nc.scalar.copy(qf[:], pq[:])
                qs = sb.tile([128, TC], F32, tag="qs", bufs=2)
                nc.sync.dma_start(qs[0:64, :], qf[64:128, :])
                nc.sync.dma_start(qs[64:128, :], qf[0:64, :])
                nc.vector.tensor_tensor(qf[:], qf[:], cos_sb[:, t0:t0 + TC], MUL)
                nc.vector.tensor_tensor(qs[:], qs[:], sin_sb[:, t0:t0 + TC], MUL)
                nc.vector.tensor_tensor(dst, qf[:], qs[:], ADD)

            qtb = None

            def proj(c):
                nonlocal qtb
                t0 = c * TC
                if c + 1 < NCHUNK:
                    load_xt(c + 1)
                qtb = sb.tile([128, HL * TC], BF16, tag="qt", bufs=1, name="qtb")
                for h in range(HL):
                    wb = sb.tile([128, KT * 128], BF16, tag="wb", bufs=2, name="wb")
                    nc.sync.dma_start(wb[:], wq_d[h])
                    pq = ps.tile([128, TC], F32, tag="pj", bufs=2, name="pq")
                    for kt in range(KT):
                        nc.tensor.matmul(
                            pq[:], wb[:, kt * 128:(kt + 1) * 128],
                            xt_tile(c, kt),
                            start=(kt == 0), stop=(kt == KT - 1))
                    rope(qtb[:, h * TC:(h + 1) * TC], pq, t0)
                for kv in range(KVL):
                    wb = sb.tile([128, KT * 128], BF16, tag="wb", bufs=2, name="wbk")
                    nc.sync.dma_start(wb[:], wk_d[kv])
                    pk = ps.tile([128, TC], F32, tag="pj", bufs=2, name="pk")
                    for kt in range(KT):
                        nc.tensor.matmul(
                            pk[:], wb[:, kt * 128:(kt + 1) * 128],
                            xt_tile(c, kt),
                            start=(kt == 0), stop=(kt == KT - 1))
                    rope(ktb[:, kv * S + t0:kv * S + t0 + TC], pk, t0)
                # V: lhsT = X^T tiles (stationary), rhs = wv (both kv heads at
                # once, 256-wide) -> V[t, c] accumulated over kt.
                wv0 = sb.tile([128, KT * 128], BF16, tag="wb", bufs=2, name="wv0")
                nc.sync.dma_start(wv0[:], wv_d[:, :KT * 128])
                wv1 = sb.tile([128, KT * 128], BF16, tag="wb", bufs=2, name="wv1")
                nc.sync.dma_start(wv1[:], wv_d[:, KT * 128:])
                for tt in range(TC // 128):
                    pv = ps.tile([128, KVCOLS], F32, tag="pj", bufs=2, name="pv")
                    for kt in range(KT):
                        lx = xt_tile(c, kt, tt * 128, 128)
                        wvh = wv0 if kt < 16 else wv1
                        nc.tensor.matmul(
                            pv[:], lx,
                            wvh[:, (kt % 16) * KVCOLS:(kt % 16 + 1) * KVCOLS],
                            start=(kt == 0), stop=(kt == KT - 1))
                    vt_idx = (t0 // 128) + tt
                    nc.scalar.copy(
                        vb[:, vt_idx * KVCOLS:(vt_idx + 1) * KVCOLS], pv[:])

            attnb = None

            def attention(c):
                nonlocal attnb
                nkt = 4 * c + 4
                attnb = sb.tile([128, HL * TC], BF16, tag="attn", bufs=1,
                                name="attnb")
                for p in range(HL // 2):
                    h0, h1 = 2 * p, 2 * p + 1
                    kv = h0 // (HL // KVL)
                    q0 = qtb[:, h0 * TC:(h0 + 1) * TC]
                    q1 = qtb[:, h1 * TC:(h1 + 1) * TC]
                    pa0 = ps.tile([128, TC], F32, tag="pa", bufs=2, name="pa0")
                    pa1 = ps.tile([128, TC], F32, tag="pa", bufs=2, name="pa1")
                    ps0 = sb.tile([128, TC], BF16, tag="ptsum", bufs=2, name="ps0")
                    ps1 = sb.tile([128, TC], BF16, tag="ptsum", bufs=2, name="ps1")

                    def emit_pa(kt, pt0, pt1):
                        vt = vb[:, kt * KVCOLS + kv * 128:
                                kt * KVCOLS + (kv + 1) * 128]
                        nc.tensor.matmul(pa0[:], vt, pt0[:],
                                         start=(kt == 0), stop=(kt == nkt - 1))
                        nc.tensor.matmul(pa1[:], vt, pt1[:],
                                         start=(kt == 0), stop=(kt == nkt - 1))

                    prev = None
                    for kt in range(nkt):
                        kts = ktb[:, kv * S + kt * 128:kv * S + (kt + 1) * 128]
                        sps0 = ps.tile([128, TC], F32, tag="s", bufs=2, name="sps0")
                        nc.tensor.matmul(sps0[:], kts, q0, start=True, stop=True)
                        sps1 = ps.tile([128, TC], F32, tag="s", bufs=2, name="sps1")
                        nc.tensor.matmul(sps1[:], kts, q1, start=True, stop=True)
                        if prev is not None:
                            emit_pa(*prev)
                        pt0 = sb.tile([128, TC], BF16, tag="pt", bufs=4, name="pt0")
                        nc.scalar.activation(pt0[:], sps0[:], EXP, scale=SCALE)
                        pt1 = sb.tile([128, TC], BF16, tag="pt", bufs=4, name="pt1")
                        nc.scalar.activation(pt1[:], sps1[:], EXP, scale=SCALE)
                        j = kt - 4 * c
                        if j >= 0:
                            msk = mask_sb[:, j * TC:(j + 1) * TC]
                            nc.vector.tensor_tensor(pt0[:], pt0[:], msk, MUL)
                            nc.vector.tensor_tensor(pt1[:], pt1[:], msk, MUL)
                        # running P^T row-block sums on Pool (for the
                        # denominator: one matmul per head instead of per kt)
                        if kt == 0:
                            nc.gpsimd.tensor_copy(ps0[:], pt0[:])
                            nc.gpsimd.tensor_copy(ps1[:], pt1[:])
                        else:
                            nc.gpsimd.tensor_tensor(ps0[:], ps0[:], pt0[:], ADD)
                            nc.gpsimd.tensor_tensor(ps1[:], ps1[:], pt1[:], ADD)
                        prev = (kt, pt0, pt1)
                    emit_pa(*prev)
                    pd0 = ps.tile([128, TC], F32, tag="pd", bufs=2, name="pd0")
                    nc.tensor.matmul(pd0[:], ones_sb[:], ps0[:],
                                     start=True, stop=True)
                    pd1 = ps.tile([128, TC], F32, tag="pd", bufs=2, name="pd1")
                    nc.tensor.matmul(pd1[:], ones_sb[:], ps1[:],
                                     start=True, stop=True)
                    rc0 = sb.tile([128, TC], F32, tag="rc", bufs=2, name="rc0")
                    nc.vector.reciprocal(rc0[:], pd0[:])
                    nc.vector.tensor_tensor(
                        attnb[:, h0 * TC:(h0 + 1) * TC], pa0[:], rc0[:], MUL)
                    rc1 = sb.tile([128, TC], F32, tag="rc", bufs=2, name="rc1")
                    nc.vector.reciprocal(rc1[:], pd1[:])
                    nc.vector.tensor_tensor(
                        attnb[:, h1 * TC:(h1 + 1) * TC], pa1[:], rc1[:], MUL)

            def outproj(c):
                """Local partial o_proj -> cci[rank, t, oc], then bf16
                ReduceScatter(add) within the 4-core group."""
                t0 = c * TC
                cci = dr.tile([TP, TC, OC], BF16, tag="cci", bufs=2, name="cci")
                cco = dr.tile([TC, OC], BF16, tag="cco", bufs=2, name="cco")
                for ocg in range(8):
                    wor = []
                    for ft in range(HL):
                        w = sb.tile([128, TC], BF16, tag="wor", bufs=16,
                                    name=f"wor{ft}")
                        nc.sync.dma_start(
                            w[:], wo_d[ft][:, ocg * TC:(ocg + 1) * TC])
                        wor.append(w)
                    for tt in range(TC // 128):
                        po = ps.tile([128, TC], F32, tag="s", bufs=2, name="po")
                        for ft in range(HL):
                            nc.tensor.matmul(
                                po[:],
                                attnb[:, ft * TC + tt * 128:ft * TC + (tt + 1) * 128],
                                wor[ft][:],
                                start=(ft == 0), stop=(ft == HL - 1))
                        ot = sb.tile([128, TC], BF16, tag="ot", bufs=4, name="ot")
                        nc.scalar.copy(ot[:], po[:])
                        nc.sync.dma_start(
                            cci[ocg // 2, tt * 128:(tt + 1) * 128,
                                (ocg % 2) * TC:(ocg % 2 + 1) * TC],
                            ot[:])
                nc.gpsimd.collective_compute(
                    "ReduceScatter", ADD,
                    replica_groups=GROUPS, ins=[cci[:]], outs=[cco[:]])
                nc.sync.dma_start(out_d[t0:t0 + TC, :], cco[:])

            # ---- chunk loop ----
            load_xt(0)
            for c in range(NCHUNK):
                proj(c)
                attention(c)
                outproj(c)

    nc.compile()
    return nc


def _get_nc():
    if "nc" not in _BUILT:
        _BUILT["nc"] = _build()
    return _BUILT["nc"]


def kernel(hidden_states, cos, sin, wq, wk, wv, wo):
    global LAST_RESULT
    nc = _get_nc()

    hidden_states = np.asarray(hidden_states, dtype=np.float32)
    cos = np.asarray(cos, dtype=np.float32)
    sin = np.asarray(sin, dtype=np.float32)
    wq = np.asarray(wq, dtype=np.float32)
    wk = np.asarray(wk, dtype=np.float32)
    wv = np.asarray(wv, dtype=np.float32)
    wo = np.asarray(wo, dtype=np.float32)

    # host-side shard prep (bf16, panel-major so every DMA is contiguous)
    xts = [np.ascontiguousarray(hidden_states[b].T).astype(B16NP)
           for b in range(B)]
    cts = [np.ascontiguousarray(cos[b].T).astype(B16NP) for b in range(B)]
    sin_eff = []
    for b in range(B):
        se = np.ascontiguousarray(sin[b].T).copy()
        se[0:64, :] *= -1.0
        sin_eff.append(se.astype(B16NP))

    masks = np.zeros((HD, 4 * TC), dtype=B16NP)
    for j in range(4):
        m = (np.arange(HD)[:, None] + 128 * j) <= np.arange(TC)[None, :]
        masks[:, j * TC:(j + 1) * TC] = m.astype(B16NP)
    onesb = np.ones((128, 128), dtype=B16NP)

    in_maps = []
    for core in range(N_CORES):
        b, r = core // TP, core % TP
        wq_s = wq[:, r * QCOLS:(r + 1) * QCOLS]
        wk_s = wk[:, r * KVCOLS:(r + 1) * KVCOLS]
        wv_s = wv[:, r * KVCOLS:(r + 1) * KVCOLS]
        wo_s = wo[r * QCOLS:(r + 1) * QCOLS, :]
        wqp = np.ascontiguousarray(
            wq_s.reshape(KT, 128, HL, 128).transpose(2, 1, 0, 3)
            .reshape(HL, 128, KT * 128)).astype(B16NP)
        wkp = np.ascontiguousarray(
            wk_s.reshape(KT, 128, KVL, 128).transpose(2, 1, 0, 3)
            .reshape(KVL, 128, KT * 128)).astype(B16NP)
        wvp = np.ascontiguousarray(
            wv_s.reshape(KT, 128, KVCOLS).transpose(1, 0, 2)
            .reshape(128, KT * KVCOLS)).astype(B16NP)
        wop = np.ascontiguousarray(
            wo_s.reshape(HL, 128, H)).astype(B16NP)
        in_maps.append({
            "xt": xts[b],
            "cos_t": cts[b],
            "sin_t": sin_eff[b],
            "wqp": wqp,
            "wkp": wkp,
            "wvp": wvp,
            "wop": wop,
            "masks": masks,
            "onesb": onesb,
        })

    res = run_bass_kernel_spmd(nc, in_maps, core_ids=list(range(N_CORES)))
    LAST_RESULT = res

    out = np.empty((B, S, H), dtype=np.float32)
    for core in range(N_CORES):
        b, r = core // TP, core % TP
        out[b, :, r * OC:(r + 1) * OC] = res.results[core]["out_t"].astype(
            np.float32)
    return out


# revision 11
# speedup vs baseline: 1.0457x; 1.0181x over previous
"""Llama GQA attention (B=2, S=2048, H=4096, 32 q heads / 8 kv heads, HD=128)
on 8 Trainium2 NeuronCores.

Sharding: DP=2 over batch x TP=4 over heads.
  core c: batch b = c // 4, tp rank r = c % 4
  - owns q heads [8r, 8r+8), kv heads [2r, 2r+2)
  - computes attention for its heads over its batch
  - o_proj: LOCAL partial over its 1024 attn features for ALL 4096 out
    cols (wo sharded by ROWS), then bf16 ReduceScatter(add) within each
    4-core batch group -> each core holds out cols [1024r, 1024(r+1)).

All inputs are pre-cast to bf16 on the host (weights in panel-major
layouts so every DMA is contiguous), so the device pipeline has zero
dtype-conversion work and no DRAM scratch round-trips.

On-chip layout is fully "transposed" ([feature, token]):
  QT/KT: [d, t] (weight tiles stationary, X^T moving)
  V:     [t, d] (X^T tiles stationary, wv moving)
  S^T[k, q] = (KT tile).T @ QT          (contraction d on partitions)
  P^T = exp(scale * S^T)                (ScalarE, fp32 PSUM -> bf16 SBUF)
  attn^T[d, q] += (V tile).T @ P^T      (contraction k-tokens on partitions)
  out[t, oc]  += (attnb tile).T @ wo    (contraction d on partitions)
Causal masking: only lower-triangle k-tiles are computed; diagonal
128x512 tiles use one of 4 static 0/1 masks (multiplied into P^T on DVE).
Softmax skips max-subtraction (scores are O(7), exp fits fp32).

Denominators: P^T tiles are accumulated on the Pool engine into a
per-head running sum (bf16), then ONE ones-matmul per (head, chunk)
produces the denominator — instead of one matmul per k-tile.

Attention processes heads in PAIRS sharing the same kv head, software-
pipelined so the exp->mask->pa chain of one head hides under the other
head's matmuls:
  per kt: [score(h0), score(h1), pa(h0, kt-1), pa(h1, kt-1)]
X^T for chunk c+1 is DMA'd (pure bf16 copy, no engine work) into a
2-slot rotation at the start of proj(c).
"""

import os
import sys

for _p in ("/opt/trn_rl_repo",):
    if _p not in sys.path:
        sys.path.append(_p)

import numpy as np
import ml_dtypes

import concourse.bacc as bacc
import concourse.mybir as mybir
import concourse.tile as tile
from concourse.bass_utils import run_bass_kernel_spmd

F32 = mybir.dt.float32
BF16 = mybir.dt.bfloat16
B16NP = ml_dtypes.bfloat16

B, S, H = 2, 2048, 4096
NH, NKV, HD = 32, 8, 128
N_CORES = 8
TP = 4
GROUPS = [[0, 1, 2, 3], [4, 5, 6, 7]]

HL = NH // TP          # 8 local q heads
KVL = NKV // TP        # 2 local kv heads
QCOLS = HL * HD        # 1024 local q cols
KVCOLS = KVL * HD      # 256 local kv cols
OC = H // TP           # 1024 out cols per core after ReduceScatter

TC = 512               # token chunk (= one attention q-block)
NCHUNK = S // TC       # 4
KT = H // 128          # 32 contraction tiles for the projections
SCALE = float(HD ** -0.5)

LAST_RESULT = None
_BUILT = {}

EXP = mybir.ActivationFunctionType.Exp
MUL = mybir.AluOpType.mult
ADD = mybir.AluOpType.add


def _build():
    nc = bacc.Bacc("TRN2", debug=False, num_devices=N_CORES)

    xt_d = nc.dram_tensor("xt", [H, S], BF16, kind="ExternalInput").ap()
    cos_d = nc.dram_tensor("cos_t", [HD, S], BF16, kind="ExternalInput").ap()
    sin_d = nc.dram_tensor("sin_t", [HD, S], BF16, kind="ExternalInput").ap()
    wq_d = nc.dram_tensor("wqp", [HL, 128, KT * 128], BF16, kind="ExternalInput").ap()
    wk_d = nc.dram_tensor("wkp", [KVL, 128, KT * 128], BF16, kind="ExternalInput").ap()
    wv_d = nc.dram_tensor("wvp", [128, KT * KVCOLS], BF16, kind="ExternalInput").ap()
    wo_d = nc.dram_tensor("wop", [HL, 128, H], BF16, kind="ExternalInput").ap()
    mask_d = nc.dram_tensor("masks", [HD, 8 * TC], BF16, kind="ExternalInput").ap()
    ones_d = nc.dram_tensor("onesb", [128, 128], BF16, kind="ExternalInput").ap()
    out_d = nc.dram_tensor("out_t", [S, OC], BF16, kind="ExternalOutput").ap()

    with tile.TileContext(nc) as tc:
        with tc.tile_pool(name="sb", bufs=1) as sb, \
             tc.tile_pool(name="ps", bufs=1, space="PSUM") as ps, \
             tc.tile_pool(name="dr", bufs=1, space="DRAM") as dr:

            # ---- persistent tiles ----
            cos_sb = sb.tile([HD, S], BF16)
            sin_sb = sb.tile([HD, S], BF16)
            mask_sb = sb.tile([HD, 8 * TC], BF16)
            ones_sb = sb.tile([128, 128], BF16)
            ktb = sb.tile([128, KVL * S], BF16)             # roped K^T
            vb = sb.tile([128, (S // 128) * KVCOLS], BF16)  # V, [t, tt*256 + d]
            # X^T bf16, 2-chunk rotation: [128, kt*(2*TC) + slot*TC + t]
            xtb = sb.tile([128, KT * 2 * TC], BF16)
            nc.sync.dma_start(cos_sb[:], cos_d[:])
            nc.sync.dma_start(sin_sb[:], sin_d[:])
            nc.sync.dma_start(mask_sb[:], mask_d[:])
            nc.sync.dma_start(ones_sb[:], ones_d[:])

            def xt_slot(c):
                return c % 2

            def load_xt(c):
                """Issue the 32 tile DMAs bringing chunk c of X^T into its
                rotation slot (pure bf16 copy, no engine work)."""
                sl = xt_slot(c)
                for kt in range(KT):
                    dst = xtb[:, kt * 2 * TC + sl * TC:kt * 2 * TC + (sl + 1) * TC]
                    nc.sync.dma_start(
                        dst, xt_d[kt * 128:(kt + 1) * 128, c * TC:(c + 1) * TC])

            def xt_tile(c, kt, lo=0, width=TC):
                sl = xt_slot(c)
                base = kt * 2 * TC + sl * TC + lo
                return xtb[:, base:base + width]

            def rope(dst, pq, t0):
                """dst (bf16 [128, TC]) = rope of pq (fp32 PSUM [128, TC])."""
                qf = sb.tile([128, TC], F32, tag="qf", bufs=2)
                nc.scalar.copy(qf[:], pq[:])
                qs = sb.tile([128, TC], F32, tag="qs", bufs=2)
                nc.sync.dma_start(qs[0:64, :], qf[64:128, :])
                nc.sync.dma_start(qs[64:128, :], qf[0:64, :])
                nc.vector.tensor_tensor(qf[:], qf[:], cos_sb[:, t0:t0 + TC], MUL)
                nc.vector.tensor_tensor(qs[:], qs[:], sin_sb[:, t0:t0 + TC], MUL)
                nc.vector.tensor_tensor(dst, qf[:], qs[:], ADD)

            qtb = None

            def proj(c):
                nonlocal qtb
                t0 = c * TC
                if c + 1 < NCHUNK:
                    load_xt(c + 1)
                qtb = sb.tile([128, HL * TC], BF16, tag="qt", bufs=1, name="qtb")
                for h in range(HL):
                    wb = sb.tile([128, KT * 128], BF16, tag="wb", bufs=2, name="wb")
                    nc.sync.dma_start(wb[:], wq_d[h])
                    pq = ps.tile([128, TC], F32, tag="pj", bufs=2, name="pq")
                    for kt in range(KT):
                        nc.tensor.matmul(
                            pq[:], wb[:, kt * 128:(kt + 1) * 128],
                            xt_tile(c, kt),
                            start=(kt == 0), stop=(kt == KT - 1))
                    rope(qtb[:, h * TC:(h + 1) * TC], pq, t0)
                for kv in range(KVL):
                    wb = sb.tile([128, KT * 128], BF16, tag="wb", bufs=2, name="wbk")
                    nc.sync.dma_start(wb[:], wk_d[kv])
                    pk = ps.tile([128, TC], F32, tag="pj", bufs=2, name="pk")
                    for kt in range(KT):
                        nc.tensor.matmul(
                            pk[:], wb[:, kt * 128:(kt + 1) * 128],
                            xt_tile(c, kt),
                            start=(kt == 0), stop=(kt == KT - 1))
                    rope(ktb[:, kv * S + t0:kv * S + t0 + TC], pk, t0)
                # V: lhsT = X^T tiles (stationary), rhs = wv (both kv heads at
                # once, 256-wide) -> V[t, c] accumulated over kt.
                wv0 = sb.tile([128, KT * 128], BF16, tag="wb", bufs=2, name="wv0")
                nc.sync.dma_start(wv0[:], wv_d[:, :KT * 128])
                wv1 = sb.tile([128, KT * 128], BF16, tag="wb", bufs=2, name="wv1")
                nc.sync.dma_start(wv1[:], wv_d[:, KT * 128:])
                for tt in range(TC // 128):
                    pv = ps.tile([128, KVCOLS], F32, tag="pj", bufs=2, name="pv")
                    for kt in range(KT):
                        lx = xt_tile(c, kt, tt * 128, 128)
                        wvh = wv0 if kt < 16 else wv1
                        nc.tensor.matmul(
                            pv[:], lx,
                            wvh[:, (kt % 16) * KVCOLS:(kt % 16 + 1) * KVCOLS],
                            start=(kt == 0), stop=(kt == KT - 1))
                    vt_idx = (t0 // 128) + tt
                    nc.scalar.copy(
                        vb[:, vt_idx * KVCOLS:(vt_idx + 1) * KVCOLS], pv[:])

            attnb = None
            _WOR_PRE = {}

            def prefetch_wor(ocg):
                """Allocate + DMA the 8 wo rhs slices for output group ocg."""
                tiles = []
                for ft in range(HL):
                    w = sb.tile([128, TC], BF16, tag="wor", bufs=16,
                                name=f"wor{ft}")
                    nc.sync.dma_start(
                        w[:], wo_d[ft][:, ocg * TC:(ocg + 1) * TC])
                    tiles.append(w)
                _WOR_PRE[ocg] = tiles
                return tiles

            def attention(c):
                """Heads processed in pairs sharing a kv head; adjacent in
                qtb, so one [128, 2*TC] score matmul + one fused exp serve
                both. pa/pd accumulate per head; masks alternate DVE/GpSimd."""
                nonlocal attnb
                nkt = 4 * c + 4
                attnb = sb.tile([128, HL * TC], BF16, tag="attn", bufs=1,
                                name="attnb")
                for p in range(HL // 2):
                    h0, h1 = 2 * p, 2 * p + 1
                    kv = h0 // (HL // KVL)
                    qpair = qtb[:, h0 * TC:(h0 + 2) * TC]
                    pa0 = ps.tile([128, TC], F32, tag="pa", bufs=2, name="pa0")
                    pa1 = ps.tile([128, TC], F32, tag="pa", bufs=2, name="pa1")
                    pd0 = ps.tile([128, TC], F32, tag="pj", bufs=2, name="pd0")
                    pd1 = ps.tile([128, TC], F32, tag="pj", bufs=2, name="pd1")

                    def emit_pa_pd(kt, pt):
                        st, sp = (kt == 0), (kt == nkt - 1)
                        vt = vb[:, kt * KVCOLS + kv * 128:
                                kt * KVCOLS + (kv + 1) * 128]
                        nc.tensor.matmul(pa0[:], vt, pt[:, :TC],
                                         start=st, stop=sp)
                        nc.tensor.matmul(pa1[:], vt, pt[:, TC:],
                                         start=st, stop=sp)
                        nc.tensor.matmul(pd0[:], ones_sb[:], pt[:, :TC],
                                         start=st, stop=sp)
                        nc.tensor.matmul(pd1[:], ones_sb[:], pt[:, TC:],
                                         start=st, stop=sp)

                    prev = None
                    for kt in range(nkt):
                        kts = ktb[:, kv * S + kt * 128:kv * S + (kt + 1) * 128]
                        sps = ps.tile([128, 2 * TC], F32, tag="s", bufs=2,
                                      name="sps")
                        # matmul out must stay within one PSUM bank: write
                        # the fused tile's halves with two 512-wide matmuls
                        # (same stationary K tile), then ONE fused exp.
                        nc.tensor.matmul(sps[:, :TC], kts,
                                         qpair[:, :TC], start=True, stop=True)
                        nc.tensor.matmul(sps[:, TC:], kts,
                                         qpair[:, TC:], start=True, stop=True)
                        if prev is not None:
                            emit_pa_pd(*prev)
                        pt = sb.tile([128, 2 * TC], BF16, tag="pt", bufs=4,
                                     name="pt")
                        nc.scalar.activation(pt[:], sps[:], EXP, scale=SCALE)
                        j = kt - 4 * c
                        if j >= 0:
                            msk = mask_sb[:, j * 2 * TC:(j + 1) * 2 * TC]
                            if kt % 2 == 0:
                                nc.vector.tensor_tensor(pt[:], pt[:], msk, MUL)
                            else:
                                nc.gpsimd.tensor_tensor(pt[:], pt[:], msk, MUL)
                        prev = (kt, pt)
                    emit_pa_pd(*prev)
                    rc0 = sb.tile([128, TC], F32, tag="rc", bufs=2, name="rc0")
                    nc.vector.reciprocal(rc0[:], pd0[:])
                    nc.vector.tensor_tensor(
                        attnb[:, h0 * TC:(h0 + 1) * TC], pa0[:], rc0[:], MUL)
                    rc1 = sb.tile([128, TC], F32, tag="rc", bufs=2, name="rc1")
                    nc.vector.reciprocal(rc1[:], pd1[:])
                    nc.vector.tensor_tensor(
                        attnb[:, h1 * TC:(h1 + 1) * TC], pa1[:], rc1[:], MUL)
                    if p == 2:
                        # hoist first o-proj weight loads under attention
                        prefetch_wor(0)

            _OUT_PENDING = []

            def outproj(c):
                """Local partial o_proj -> cci[rank, t, oc], then bf16
                ReduceScatter(add) within the 4-core group. The cco->out_t
                DMAs are deferred to the end of the program so no in-order
                DMA queue mid-stream carries a descriptor that has to wait
                for a collective. The last chunk's RS is split in two
                halves (even / odd output groups) so the first RS overlaps
                the second half of the out-projection matmuls."""
                t0 = c * TC
                last = (c == NCHUNK - 1)
                if last:
                    ccis = [dr.tile([TP, TC, OC // 2], BF16, tag="cci",
                                    bufs=2, name=f"cci{h}") for h in range(2)]
                    order = [0, 2, 4, 6, 1, 3, 5, 7]
                else:
                    ccis = [dr.tile([TP, TC, OC], BF16, tag="cci", bufs=2,
                                    name="cci")]
                    order = list(range(8))

                def emit_rs(half):
                    if last:
                        cco = dr.tile([TC, OC // 2], BF16, tag="cco", bufs=5,
                                      name=f"cco{half}")
                        dst = out_d[t0:t0 + TC,
                                    half * (OC // 2):(half + 1) * (OC // 2)]
                    else:
                        cco = dr.tile([TC, OC], BF16, tag="cco", bufs=5,
                                      name="cco")
                        dst = out_d[t0:t0 + TC, :]
                    nc.gpsimd.collective_compute(
                        "ReduceScatter", ADD,
                        replica_groups=GROUPS, ins=[ccis[half][:]],
                        outs=[cco[:]])
                    _OUT_PENDING.append((dst, cco))

                for i, ocg in enumerate(order):
                    wor = _WOR_PRE.pop(ocg, None) or prefetch_wor(ocg)
                    if i + 1 < 8:
                        prefetch_wor(order[i + 1])
                    if last:
                        cci, ocs = ccis[ocg % 2], 0
                    else:
                        cci, ocs = ccis[0], (ocg % 2) * TC
                    for tt in range(TC // 128):
                        po = ps.tile([128, TC], F32, tag="s", bufs=2, name="po")
                        for ft in range(HL):
                            nc.tensor.matmul(
                                po[:],
                                attnb[:, ft * TC + tt * 128:ft * TC + (tt + 1) * 128],
                                wor[ft][:],
                                start=(ft == 0), stop=(ft == HL - 1))
                        ot = sb.tile([128, TC], BF16, tag="ot", bufs=4, name="ot")
                        nc.scalar.copy(ot[:], po[:])
                        nc.sync.dma_start(
                            cci[ocg // 2, tt * 128:(tt + 1) * 128,
                                ocs:ocs + TC],
                            ot[:])
                    if last and i == 3:
                        emit_rs(0)
                if last:
                    emit_rs(1)
                else:
                    emit_rs(0)

            # ---- chunk loop ----
            load_xt(0)
            for c in range(NCHUNK):
                proj(c)
                attention(c)
                outproj(c)
            for dst, cco in _OUT_PENDING:
                nc.sync.dma_start(dst, cco[:])

    nc.compile()
    return nc


def _get_nc():
    if "nc" not in _BUILT:
        _BUILT["nc"] = _build()
    return _BUILT["nc"]


def kernel(hidden_states, cos, sin, wq, wk, wv, wo):
    global LAST_RESULT
    nc = _get_nc()

    hidden_states = np.asarray(hidden_states, dtype=np.float32)
    cos = np.asarray(cos, dtype=np.float32)
    sin = np.asarray(sin, dtype=np.float32)
    wq = np.asarray(wq, dtype=np.float32)
    wk = np.asarray(wk, dtype=np.float32)
    wv = np.asarray(wv, dtype=np.float32)
    wo = np.asarray(wo, dtype=np.float32)

    # host-side shard prep (bf16, panel-major so every DMA is contiguous)
    xts = [np.ascontiguousarray(hidden_states[b].T).astype(B16NP)
           for b in range(B)]
    cts = [np.ascontiguousarray(cos[b].T).astype(B16NP) for b in range(B)]
    sin_eff = []
    for b in range(B):
        se = np.ascontiguousarray(sin[b].T).copy()
        se[0:64, :] *= -1.0
        sin_eff.append(se.astype(B16NP))

    # fused pair masks: each diagonal mask duplicated for the head pair
    masks = np.zeros((HD, 8 * TC), dtype=B16NP)
    for j in range(4):
        m = ((np.arange(HD)[:, None] + 128 * j)
             <= np.arange(TC)[None, :]).astype(B16NP)
        masks[:, j * 2 * TC:j * 2 * TC + TC] = m
        masks[:, j * 2 * TC + TC:(j + 1) * 2 * TC] = m
    onesb = np.ones((128, 128), dtype=B16NP)

    in_maps = []
    for core in range(N_CORES):
        b, r = core // TP, core % TP
        wq_s = wq[:, r * QCOLS:(r + 1) * QCOLS]
        wk_s = wk[:, r * KVCOLS:(r + 1) * KVCOLS]
        wv_s = wv[:, r * KVCOLS:(r + 1) * KVCOLS]
        wo_s = wo[r * QCOLS:(r + 1) * QCOLS, :]
        wqp = np.ascontiguousarray(
            wq_s.reshape(KT, 128, HL, 128).transpose(2, 1, 0, 3)
            .reshape(HL, 128, KT * 128)).astype(B16NP)
        wkp = np.ascontiguousarray(
            wk_s.reshape(KT, 128, KVL, 128).transpose(2, 1, 0, 3)
            .reshape(KVL, 128, KT * 128)).astype(B16NP)
        wvp = np.ascontiguousarray(
            wv_s.reshape(KT, 128, KVCOLS).transpose(1, 0, 2)
            .reshape(128, KT * KVCOLS)).astype(B16NP)
        wop = np.ascontiguousarray(
            wo_s.reshape(HL, 128, H)).astype(B16NP)
        in_maps.append({
            "xt": xts[b],
            "cos_t": cts[b],
            "sin_t": sin_eff[b],
            "wqp": wqp,
            "wkp": wkp,
            "wvp": wvp,
            "wop": wop,
            "masks": masks,
            "onesb": onesb,
        })

    res = run_bass_kernel_spmd(nc, in_maps, core_ids=list(range(N_CORES)))
    LAST_RESULT = res

    out = np.empty((B, S, H), dtype=np.float32)
    for core in range(N_CORES):
        b, r = core // TP, core % TP
        out[b, :, r * OC:(r + 1) * OC] = res.results[core]["out_t"].astype(
            np.float32)
    return out


# revision 15
# speedup vs baseline: 1.0487x; 1.0029x over previous
"""Llama GQA attention (B=2, S=2048, H=4096, 32 q heads / 8 kv heads, HD=128)
on 8 Trainium2 NeuronCores.

Sharding: DP=2 over batch x TP=4 over heads.
  core c: batch b = c // 4, tp rank r = c % 4
  - owns q heads [8r, 8r+8), kv heads [2r, 2r+2)
  - computes attention for its heads over its batch
  - o_proj: LOCAL partial over its 1024 attn features for ALL 4096 out
    cols (wo sharded by ROWS), then bf16 ReduceScatter(add) within each
    4-core batch group -> each core holds out cols [1024r, 1024(r+1)).

All inputs are pre-cast to bf16 on the host (weights in panel-major
layouts so every DMA is contiguous), so the device pipeline has zero
dtype-conversion work and no DRAM scratch round-trips.

On-chip layout is fully "transposed" ([feature, token]):
  QT/KT: [d, t] (weight tiles stationary, X^T moving)
  V:     [t, d] (X^T tiles stationary, wv moving)
  S^T[k, q] = (KT tile).T @ QT          (contraction d on partitions)
  P^T = exp(scale * S^T)                (ScalarE, fp32 PSUM -> bf16 SBUF)
  attn^T[d, q] += (V tile).T @ P^T      (contraction k-tokens on partitions)
  out[t, oc]  += (attnb tile).T @ wo    (contraction d on partitions)
Causal masking: only lower-triangle k-tiles are computed; diagonal
128x512 tiles use one of 4 static 0/1 masks (multiplied into P^T on DVE).
Softmax skips max-subtraction (scores are O(7), exp fits fp32).

Denominators: P^T tiles are accumulated on the Pool engine into a
per-head running sum (bf16), then ONE ones-matmul per (head, chunk)
produces the denominator — instead of one matmul per k-tile.

Attention processes heads in PAIRS sharing the same kv head, software-
pipelined so the exp->mask->pa chain of one head hides under the other
head's matmuls:
  per kt: [score(h0), score(h1), pa(h0, kt-1), pa(h1, kt-1)]
X^T for chunk c+1 is DMA'd (pure bf16 copy, no engine work) into a
2-slot rotation at the start of proj(c).
"""

import os
import sys

for _p in ("/opt/trn_rl_repo",):
    if _p not in sys.path:
        sys.path.append(_p)

import numpy as np
import ml_dtypes

import concourse.bacc as bacc
import concourse.mybir as mybir
import concourse.tile as tile
from concourse.bass_utils import run_bass_kernel_spmd

F32 = mybir.dt.float32
BF16 = mybir.dt.bfloat16
B16NP = ml_dtypes.bfloat16

B, S, H = 2, 2048, 4096
NH, NKV, HD = 32, 8, 128
N_CORES = 8
TP = 4
GROUPS = [[0, 1, 2, 3], [4, 5, 6, 7]]

HL = NH // TP          # 8 local q heads
KVL = NKV // TP        # 2 local kv heads
QCOLS = HL * HD        # 1024 local q cols
KVCOLS = KVL * HD      # 256 local kv cols
OC = H // TP           # 1024 out cols per core after ReduceScatter

TC = 512               # token chunk (= one attention q-block)
NCHUNK = S // TC       # 4
KT = H // 128          # 32 contraction tiles for the projections
SCALE = float(HD ** -0.5)

LAST_RESULT = None
_BUILT = {}

EXP = mybir.ActivationFunctionType.Exp
MUL = mybir.AluOpType.mult
ADD = mybir.AluOpType.add


def _build():
    nc = bacc.Bacc("TRN2", debug=False, num_devices=N_CORES)

    xt_d = nc.dram_tensor("xt", [H, S], BF16, kind="ExternalInput").ap()
    cos_d = nc.dram_tensor("cos_t", [HD, S], BF16, kind="ExternalInput").ap()
    sin_d = nc.dram_tensor("sin_t", [HD, S], BF16, kind="ExternalInput").ap()
    wq_d = nc.dram_tensor("wqp", [HL, 128, KT * 128], BF16, kind="ExternalInput").ap()
    wk_d = nc.dram_tensor("wkp", [KVL, 128, KT * 128], BF16, kind="ExternalInput").ap()
    wv_d = nc.dram_tensor("wvp", [128, KT * KVCOLS], BF16, kind="ExternalInput").ap()
    wo_d = nc.dram_tensor("wop", [HL, 128, H], BF16, kind="ExternalInput").ap()
    mask_d = nc.dram_tensor("masks", [HD, 8 * TC], BF16, kind="ExternalInput").ap()
    ones_d = nc.dram_tensor("onesb", [128, 128], BF16, kind="ExternalInput").ap()
    out_d = nc.dram_tensor("out_t", [S, OC], BF16, kind="ExternalOutput").ap()

    with tile.TileContext(nc) as tc:
        with tc.tile_pool(name="sb", bufs=1) as sb, \
             tc.tile_pool(name="ps", bufs=1, space="PSUM") as ps, \
             tc.tile_pool(name="dr", bufs=1, space="DRAM") as dr:

            # ---- persistent tiles ----
            cos_sb = sb.tile([HD, S], BF16)
            sin_sb = sb.tile([HD, S], BF16)
            mask_sb = sb.tile([HD, 8 * TC], BF16)
            ones_sb = sb.tile([128, 128], BF16)
            ktb = sb.tile([128, KVL * S], BF16)             # roped K^T
            vb = sb.tile([128, (S // 128) * KVCOLS], BF16)  # V, [t, tt*256 + d]
            # X^T bf16, 2-chunk rotation: [128, kt*(2*TC) + slot*TC + t]
            xtb = sb.tile([128, KT * 2 * TC], BF16)
            nc.sync.dma_start(cos_sb[:], cos_d[:])
            nc.sync.dma_start(sin_sb[:], sin_d[:])
            nc.sync.dma_start(mask_sb[:], mask_d[:])
            nc.sync.dma_start(ones_sb[:], ones_d[:])

            def xt_slot(c):
                return c % 2

            def load_xt(c):
                """Issue the 32 tile DMAs bringing chunk c of X^T into its
                rotation slot (pure bf16 copy, no engine work)."""
                sl = xt_slot(c)
                for kt in range(KT):
                    dst = xtb[:, kt * 2 * TC + sl * TC:kt * 2 * TC + (sl + 1) * TC]
                    nc.sync.dma_start(
                        dst, xt_d[kt * 128:(kt + 1) * 128, c * TC:(c + 1) * TC])

            def xt_tile(c, kt, lo=0, width=TC):
                sl = xt_slot(c)
                base = kt * 2 * TC + sl * TC + lo
                return xtb[:, base:base + width]

            def rope(dst, pq, t0):
                """dst (bf16 [128, TC]) = rope of pq (fp32 PSUM [128, TC])."""
                qf = sb.tile([128, TC], F32, tag="qf", bufs=2)
                nc.scalar.copy(qf[:], pq[:])
                qs = sb.tile([128, TC], F32, tag="qs", bufs=2)
                nc.sync.dma_start(qs[0:64, :], qf[64:128, :])
                nc.sync.dma_start(qs[64:128, :], qf[0:64, :])
                nc.vector.tensor_tensor(qf[:], qf[:], cos_sb[:, t0:t0 + TC], MUL)
                nc.vector.tensor_tensor(qs[:], qs[:], sin_sb[:, t0:t0 + TC], MUL)
                nc.vector.tensor_tensor(dst, qf[:], qs[:], ADD)

            qtb = None

            def proj(c):
                nonlocal qtb
                t0 = c * TC
                if c + 1 < NCHUNK:
                    load_xt(c + 1)
                qtb = sb.tile([128, HL * TC], BF16, tag="qt", bufs=1, name="qtb")
                for h in range(HL):
                    wb = sb.tile([128, KT * 128], BF16, tag="wb", bufs=2, name="wb")
                    nc.sync.dma_start(wb[:], wq_d[h])
                    pq = ps.tile([128, TC], F32, tag="pj", bufs=2, name="pq")
                    for kt in range(KT):
                        nc.tensor.matmul(
                            pq[:], wb[:, kt * 128:(kt + 1) * 128],
                            xt_tile(c, kt),
                            start=(kt == 0), stop=(kt == KT - 1))
                    rope(qtb[:, h * TC:(h + 1) * TC], pq, t0)
                for kv in range(KVL):
                    wb = sb.tile([128, KT * 128], BF16, tag="wb", bufs=2, name="wbk")
                    nc.sync.dma_start(wb[:], wk_d[kv])
                    pk = ps.tile([128, TC], F32, tag="pj", bufs=2, name="pk")
                    for kt in range(KT):
                        nc.tensor.matmul(
                            pk[:], wb[:, kt * 128:(kt + 1) * 128],
                            xt_tile(c, kt),
                            start=(kt == 0), stop=(kt == KT - 1))
                    rope(ktb[:, kv * S + t0:kv * S + t0 + TC], pk, t0)
                # V: lhsT = X^T tiles (stationary), rhs = wv (both kv heads at
                # once, 256-wide) -> V[t, c] accumulated over kt.
                wv0 = sb.tile([128, KT * 128], BF16, tag="wb", bufs=2, name="wv0")
                nc.sync.dma_start(wv0[:], wv_d[:, :KT * 128])
                wv1 = sb.tile([128, KT * 128], BF16, tag="wb", bufs=2, name="wv1")
                nc.sync.dma_start(wv1[:], wv_d[:, KT * 128:])
                for tt in range(TC // 128):
                    pv = ps.tile([128, KVCOLS], F32, tag="pj", bufs=2, name="pv")
                    for kt in range(KT):
                        lx = xt_tile(c, kt, tt * 128, 128)
                        wvh = wv0 if kt < 16 else wv1
                        nc.tensor.matmul(
                            pv[:], lx,
                            wvh[:, (kt % 16) * KVCOLS:(kt % 16 + 1) * KVCOLS],
                            start=(kt == 0), stop=(kt == KT - 1))
                    vt_idx = (t0 // 128) + tt
                    nc.scalar.copy(
                        vb[:, vt_idx * KVCOLS:(vt_idx + 1) * KVCOLS], pv[:])

            attnb = None
            _WOR_PRE = {}

            def prefetch_wor(ocg):
                """Allocate + DMA the 8 wo rhs slices for output group ocg."""
                tiles = []
                for ft in range(HL):
                    w = sb.tile([128, TC], BF16, tag="wor", bufs=16,
                                name=f"wor{ft}")
                    nc.sync.dma_start(
                        w[:], wo_d[ft][:, ocg * TC:(ocg + 1) * TC])
                    tiles.append(w)
                _WOR_PRE[ocg] = tiles
                return tiles

            def attention(c):
                """Heads processed in pairs sharing a kv head; adjacent in
                qtb, so one [128, 2*TC] score matmul + one fused exp serve
                both. pa/pd accumulate per head; masks alternate DVE/GpSimd."""
                nonlocal attnb
                nkt = 4 * c + 4
                attnb = sb.tile([128, HL * TC], BF16, tag="attn", bufs=1,
                                name="attnb")
                for p in range(HL // 2):
                    h0, h1 = 2 * p, 2 * p + 1
                    kv = h0 // (HL // KVL)
                    qpair = qtb[:, h0 * TC:(h0 + 2) * TC]
                    pa0 = ps.tile([128, TC], F32, tag="pa", bufs=2, name="pa0")
                    pa1 = ps.tile([128, TC], F32, tag="pa", bufs=2, name="pa1")
                    pd0 = ps.tile([128, TC], F32, tag="pj", bufs=2, name="pd0")
                    pd1 = ps.tile([128, TC], F32, tag="pj", bufs=2, name="pd1")

                    def emit_pa_pd(kt, pt):
                        st, sp = (kt == 0), (kt == nkt - 1)
                        vt = vb[:, kt * KVCOLS + kv * 128:
                                kt * KVCOLS + (kv + 1) * 128]
                        nc.tensor.matmul(pa0[:], vt, pt[:, :TC],
                                         start=st, stop=sp)
                        nc.tensor.matmul(pa1[:], vt, pt[:, TC:],
                                         start=st, stop=sp)
                        nc.tensor.matmul(pd0[:], ones_sb[:], pt[:, :TC],
                                         start=st, stop=sp)
                        nc.tensor.matmul(pd1[:], ones_sb[:], pt[:, TC:],
                                         start=st, stop=sp)

                    prev = None
                    for kt in range(nkt):
                        kts = ktb[:, kv * S + kt * 128:kv * S + (kt + 1) * 128]
                        sps = ps.tile([128, 2 * TC], F32, tag="s", bufs=2,
                                      name="sps")
                        # matmul out must stay within one PSUM bank: write
                        # the fused tile's halves with two 512-wide matmuls
                        # (same stationary K tile), then ONE fused exp.
                        nc.tensor.matmul(sps[:, :TC], kts,
                                         qpair[:, :TC], start=True, stop=True)
                        nc.tensor.matmul(sps[:, TC:], kts,
                                         qpair[:, TC:], start=True, stop=True)
                        if prev is not None:
                            emit_pa_pd(*prev)
                        pt = sb.tile([128, 2 * TC], BF16, tag="pt", bufs=4,
                                     name="pt")
                        nc.scalar.activation(pt[:], sps[:], EXP, scale=SCALE)
                        j = kt - 4 * c
                        if j >= 0:
                            # always DVE: GpSimd must stay free of compute so
                            # a collective's completion wait blocks nothing
                            msk = mask_sb[:, j * 2 * TC:(j + 1) * 2 * TC]
                            nc.vector.tensor_tensor(pt[:], pt[:], msk, MUL)
                        prev = (kt, pt)
                    emit_pa_pd(*prev)
                    rc0 = sb.tile([128, TC], F32, tag="rc", bufs=2, name="rc0")
                    nc.vector.reciprocal(rc0[:], pd0[:])
                    nc.vector.tensor_tensor(
                        attnb[:, h0 * TC:(h0 + 1) * TC], pa0[:], rc0[:], MUL)
                    rc1 = sb.tile([128, TC], F32, tag="rc", bufs=2, name="rc1")
                    nc.vector.reciprocal(rc1[:], pd1[:])
                    nc.vector.tensor_tensor(
                        attnb[:, h1 * TC:(h1 + 1) * TC], pa1[:], rc1[:], MUL)
                    if p == 2:
                        # hoist first o-proj weight loads under attention
                        prefetch_wor(0)

            def outproj(c):
                """Local partial o_proj -> cci[rank, t, oc], then bf16
                ReduceScatter(add) within the 4-core group. The cco->out_t
                DMAs are deferred to the end of the program so no in-order
                DMA queue mid-stream carries a descriptor that has to wait
                for a collective. The last chunk's RS is split in two
                halves (even / odd output groups) so the first RS overlaps
                the second half of the out-projection matmuls."""
                t0 = c * TC
                last = (c == NCHUNK - 1)
                if last:
                    ccis = [dr.tile([TP, TC, OC // 2], BF16, tag="cci",
                                    bufs=2, name=f"cci{h}") for h in range(2)]
                    order = [0, 2, 4, 6, 1, 3, 5, 7]
                else:
                    ccis = [dr.tile([TP, TC, OC], BF16, tag="cci", bufs=2,
                                    name="cci")]
                    order = list(range(8))

                def emit_rs(half):
                    if last:
                        cco = dr.tile([TC, OC // 2], BF16, tag="cco", bufs=5,
                                      name=f"cco{half}")
                        dst = out_d[t0:t0 + TC,
                                    half * (OC // 2):(half + 1) * (OC // 2)]
                    else:
                        cco = dr.tile([TC, OC], BF16, tag="cco", bufs=5,
                                      name="cco")
                        dst = out_d[t0:t0 + TC, :]
                    nc.gpsimd.collective_compute(
                        "ReduceScatter", ADD,
                        replica_groups=GROUPS, ins=[ccis[half][:]],
                        outs=[cco[:]])
                    # cco->out on the GpSimd stream: its wait for the RS
                    # blocks only GpSimd, which hosts no other work. A
                    # sync-engine DMA here would stall ALL DMA dispatch
                    # for the collective's full latency.
                    nc.gpsimd.dma_start(dst, cco[:])

                for i, ocg in enumerate(order):
                    wor = _WOR_PRE.pop(ocg, None) or prefetch_wor(ocg)
                    if i + 1 < 8:
                        prefetch_wor(order[i + 1])
                    if last:
                        cci, ocs = ccis[ocg % 2], 0
                    else:
                        cci, ocs = ccis[0], (ocg % 2) * TC
                    for tt in range(TC // 128):
                        po = ps.tile([128, TC], F32, tag="s", bufs=2, name="po")
                        for ft in range(HL):
                            nc.tensor.matmul(
                                po[:],
                                attnb[:, ft * TC + tt * 128:ft * TC + (tt + 1) * 128],
                                wor[ft][:],
                                start=(ft == 0), stop=(ft == HL - 1))
                        ot = sb.tile([128, TC], BF16, tag="ot", bufs=4, name="ot")
                        nc.scalar.copy(ot[:], po[:])
                        nc.sync.dma_start(
                            cci[ocg // 2, tt * 128:(tt + 1) * 128,
                                ocs:ocs + TC],
                            ot[:])
                    if last and i == 3:
                        emit_rs(0)
                if last:
                    emit_rs(1)
                else:
                    emit_rs(0)

            # ---- chunk loop ----
            load_xt(0)
            for c in range(NCHUNK):
                proj(c)
                attention(c)
                outproj(c)

    nc.compile()
    return nc


def _get_nc():
    if "nc" not in _BUILT:
        _BUILT["nc"] = _build()
    return _BUILT["nc"]


def kernel(hidden_states, cos, sin, wq, wk, wv, wo):
    global LAST_RESULT
    nc = _get_nc()

    hidden_states = np.asarray(hidden_states, dtype=np.float32)
    cos = np.asarray(cos, dtype=np.float32)
    sin = np.asarray(sin, dtype=np.float32)
    wq = np.asarray(wq, dtype=np.float32)
    wk = np.asarray(wk, dtype=np.float32)
    wv = np.asarray(wv, dtype=np.float32)
    wo = np.asarray(wo, dtype=np.float32)

    # host-side shard prep (bf16, panel-major so every DMA is contiguous)
    xts = [np.ascontiguousarray(hidden_states[b].T).astype(B16NP)
           for b in range(B)]
    cts = [np.ascontiguousarray(cos[b].T).astype(B16NP) for b in range(B)]
    sin_eff = []
    for b in range(B):
        se = np.ascontiguousarray(sin[b].T).copy()
        se[0:64, :] *= -1.0
        sin_eff.append(se.astype(B16NP))

    # fused pair masks: each diagonal mask duplicated for the head pair
    masks = np.zeros((HD, 8 * TC), dtype=B16NP)
    for j in range(4):
        m = ((np.arange(HD)[:, None] + 128 * j)
             <= np.arange(TC)[None, :]).astype(B16NP)
        masks[:, j * 2 * TC:j * 2 * TC + TC] = m
        masks[:, j * 2 * TC + TC:(j + 1) * 2 * TC] = m
    onesb = np.ones((128, 128), dtype=B16NP)

    in_maps = []
    for core in range(N_CORES):
        b, r = core // TP, core % TP
        wq_s = wq[:, r * QCOLS:(r + 1) * QCOLS]
        wk_s = wk[:, r * KVCOLS:(r + 1) * KVCOLS]
        wv_s = wv[:, r * KVCOLS:(r + 1) * KVCOLS]
        wo_s = wo[r * QCOLS:(r + 1) * QCOLS, :]
        wqp = np.ascontiguousarray(
            wq_s.reshape(KT, 128, HL, 128).transpose(2, 1, 0, 3)
            .reshape(HL, 128, KT * 128)).astype(B16NP)
        wkp = np.ascontiguousarray(
            wk_s.reshape(KT, 128, KVL, 128).transpose(2, 1, 0, 3)
            .reshape(KVL, 128, KT * 128)).astype(B16NP)
        wvp = np.ascontiguousarray(
            wv_s.reshape(KT, 128, KVCOLS).transpose(1, 0, 2)
            .reshape(128, KT * KVCOLS)).astype(B16NP)
        wop = np.ascontiguousarray(
            wo_s.reshape(HL, 128, H)).astype(B16NP)
        in_maps.append({
            "xt": xts[b],
            "cos_t": cts[b],
            "sin_t": sin_eff[b],
            "wqp": wqp,
            "wkp": wkp,
            "wvp": wvp,
            "wop": wop,
            "masks": masks,
            "onesb": onesb,
        })

    res = run_bass_kernel_spmd(nc, in_maps, core_ids=list(range(N_CORES)))
    LAST_RESULT = res

    out = np.empty((B, S, H), dtype=np.float32)
    for core in range(N_CORES):
        b, r = core // TP, core % TP
        out[b, :, r * OC:(r + 1) * OC] = res.results[core]["out_t"].astype(
            np.float32)
    return out


# revision 18
# speedup vs baseline: 1.1337x; 1.0811x over previous
"""Llama GQA attention (B=2, S=2048, H=4096, 32 q heads / 8 kv heads, HD=128)
on 8 Trainium2 NeuronCores.

Sharding: DP=2 over batch x TP=4 over heads.
  core c: batch b = c // 4, tp rank r = c % 4
  - owns q heads [8r, 8r+8), kv heads [2r, 2r+2)
  - computes attention for its heads over its batch
  - o_proj: LOCAL partial over its 1024 attn features for ALL 4096 out
    cols (wo sharded by ROWS), then bf16 ReduceScatter(add) within each
    4-core batch group -> each core holds out cols [1024r, 1024(r+1)).

All inputs are pre-cast to bf16 on the host (weights in panel-major
layouts so every DMA is contiguous), so the device pipeline has zero
dtype-conversion work and no DRAM scratch round-trips.

On-chip layout is fully "transposed" ([feature, token]):
  QT/KT: [d, t] (weight tiles stationary, X^T moving)
  V:     [t, d] (X^T tiles stationary, wv moving)
  S^T[k, q] = (KT tile).T @ QT          (contraction d on partitions)
  P^T = exp(scale * S^T)                (ScalarE, fp32 PSUM -> bf16 SBUF)
  attn^T[d, q] += (V tile).T @ P^T      (contraction k-tokens on partitions)
  out[t, oc]  += (attnb tile).T @ wo    (contraction d on partitions)
Causal masking: only lower-triangle k-tiles are computed; diagonal
128x512 tiles use one of 4 static 0/1 masks (multiplied into P^T on DVE).
Softmax skips max-subtraction (scores are O(7), exp fits fp32).

Denominators: P^T tiles are accumulated on the Pool engine into a
per-head running sum (bf16), then ONE ones-matmul per (head, chunk)
produces the denominator — instead of one matmul per k-tile.

Attention processes heads in PAIRS sharing the same kv head, software-
pipelined so the exp->mask->pa chain of one head hides under the other
head's matmuls:
  per kt: [score(h0), score(h1), pa(h0, kt-1), pa(h1, kt-1)]
X^T for chunk c+1 is DMA'd (pure bf16 copy, no engine work) into a
2-slot rotation at the start of proj(c).
"""

import os
import sys

for _p in ("/opt/trn_rl_repo",):
    if _p not in sys.path:
        sys.path.append(_p)

import numpy as np
import ml_dtypes

import concourse.bacc as bacc
import concourse.mybir as mybir
import concourse.tile as tile
from concourse.bass_utils import run_bass_kernel_spmd

F32 = mybir.dt.float32
BF16 = mybir.dt.bfloat16
B16NP = ml_dtypes.bfloat16

B, S, H = 2, 2048, 4096
NH, NKV, HD = 32, 8, 128
N_CORES = 8
TP = 4
GROUPS = [[0, 1, 2, 3], [4, 5, 6, 7]]

HL = NH // TP          # 8 local q heads
KVL = NKV // TP        # 2 local kv heads
QCOLS = HL * HD        # 1024 local q cols
KVCOLS = KVL * HD      # 256 local kv cols
OC = H // TP           # 1024 out cols per core after ReduceScatter

TC = 512               # token chunk (= one attention q-block)
NCHUNK = S // TC       # 4
KT = H // 128          # 32 contraction tiles for the projections
SCALE = float(HD ** -0.5)

LAST_RESULT = None
_BUILT = {}

EXP = mybir.ActivationFunctionType.Exp
MUL = mybir.AluOpType.mult
ADD = mybir.AluOpType.add


def _build():
    nc = bacc.Bacc("TRN2", debug=False, num_devices=N_CORES)

    xt_d = nc.dram_tensor("xt", [H, S], BF16, kind="ExternalInput").ap()
    cos_d = nc.dram_tensor("cos_t", [HD, S], BF16, kind="ExternalInput").ap()
    sin_d = nc.dram_tensor("sin_t", [HD, S], BF16, kind="ExternalInput").ap()
    wq_d = nc.dram_tensor("wqp", [HL, 128, KT * 128], BF16, kind="ExternalInput").ap()
    wk_d = nc.dram_tensor("wkp", [KVL, 128, KT * 128], BF16, kind="ExternalInput").ap()
    wv_d = nc.dram_tensor("wvp", [128, KT * KVCOLS], BF16, kind="ExternalInput").ap()
    wo_d = nc.dram_tensor("wop", [HL, 128, H], BF16, kind="ExternalInput").ap()
    mask_d = nc.dram_tensor("masks", [HD, 8 * TC], BF16, kind="ExternalInput").ap()
    ones_d = nc.dram_tensor("onesb", [128, 128], BF16, kind="ExternalInput").ap()
    out_d = nc.dram_tensor("out_t", [S, OC], BF16, kind="ExternalOutput").ap()

    with tile.TileContext(nc) as tc:
        with tc.tile_pool(name="sb", bufs=1) as sb, \
             tc.tile_pool(name="ps", bufs=1, space="PSUM") as ps, \
             tc.tile_pool(name="dr", bufs=1, space="DRAM") as dr:

            # ---- persistent tiles ----
            cos_sb = sb.tile([HD, S], BF16)
            sin_sb = sb.tile([HD, S], BF16)
            mask_sb = sb.tile([HD, 8 * TC], BF16)
            ones_sb = sb.tile([128, 128], BF16)
            ktb = sb.tile([128, KVL * S], BF16)             # roped K^T
            vb = sb.tile([128, (S // 128) * KVCOLS], BF16)  # V, [t, tt*256 + d]
            # X^T bf16, 2-chunk rotation: [128, kt*(2*TC) + slot*TC + t]
            xtb = sb.tile([128, KT * 2 * TC], BF16)
            nc.sync.dma_start(cos_sb[:], cos_d[:])
            nc.sync.dma_start(sin_sb[:], sin_d[:])
            nc.sync.dma_start(mask_sb[:], mask_d[:])
            nc.sync.dma_start(ones_sb[:], ones_d[:])

            def xt_slot(c):
                return c % 2

            def load_xt(c):
                """ONE batched DMA bringing chunk c of X^T into its rotation
                slot. (Each DMA instruction costs ~0.7us of serial dispatch
                on the Sync sequencer — batch aggressively.)"""
                sl = xt_slot(c)
                dst = xtb.rearrange("p (kt s t) -> p kt s t", s=2, t=TC)[:, :, sl, :]
                src = xt_d[:, c * TC:(c + 1) * TC].rearrange(
                    "(kt p) t -> p kt t", p=128)
                nc.sync.dma_start(dst, src)

            def xt_tile(c, kt, lo=0, width=TC):
                sl = xt_slot(c)
                base = kt * 2 * TC + sl * TC + lo
                return xtb[:, base:base + width]

            def rope(dst, pq, t0):
                """dst (bf16 [128, TC]) = rope of pq (fp32 PSUM [128, TC])."""
                qf = sb.tile([128, TC], F32, tag="qf", bufs=2)
                nc.scalar.copy(qf[:], pq[:])
                qs = sb.tile([128, TC], F32, tag="qs", bufs=2)
                nc.sync.dma_start(qs[0:64, :], qf[64:128, :])
                nc.sync.dma_start(qs[64:128, :], qf[0:64, :])
                nc.vector.tensor_tensor(qf[:], qf[:], cos_sb[:, t0:t0 + TC], MUL)
                nc.vector.tensor_tensor(qs[:], qs[:], sin_sb[:, t0:t0 + TC], MUL)
                nc.vector.tensor_tensor(dst, qf[:], qs[:], ADD)

            qtb = None

            def proj(c):
                nonlocal qtb
                t0 = c * TC
                if c + 1 < NCHUNK:
                    load_xt(c + 1)
                qtb = sb.tile([128, HL * TC], BF16, tag="qt", bufs=1, name="qtb")
                for h in range(HL):
                    wb = sb.tile([128, KT * 128], BF16, tag="wb", bufs=2, name="wb")
                    nc.sync.dma_start(wb[:], wq_d[h])
                    pq = ps.tile([128, TC], F32, tag="pj", bufs=2, name="pq")
                    for kt in range(KT):
                        nc.tensor.matmul(
                            pq[:], wb[:, kt * 128:(kt + 1) * 128],
                            xt_tile(c, kt),
                            start=(kt == 0), stop=(kt == KT - 1))
                    rope(qtb[:, h * TC:(h + 1) * TC], pq, t0)
                for kv in range(KVL):
                    wb = sb.tile([128, KT * 128], BF16, tag="wb", bufs=2, name="wbk")
                    nc.sync.dma_start(wb[:], wk_d[kv])
                    pk = ps.tile([128, TC], F32, tag="pj", bufs=2, name="pk")
                    for kt in range(KT):
                        nc.tensor.matmul(
                            pk[:], wb[:, kt * 128:(kt + 1) * 128],
                            xt_tile(c, kt),
                            start=(kt == 0), stop=(kt == KT - 1))
                    rope(ktb[:, kv * S + t0:kv * S + t0 + TC], pk, t0)
                # V: lhsT = X^T tiles (stationary), rhs = wv (both kv heads at
                # once, 256-wide) -> V[t, c] accumulated over kt.
                wv0 = sb.tile([128, KT * 128], BF16, tag="wb", bufs=2, name="wv0")
                nc.sync.dma_start(wv0[:], wv_d[:, :KT * 128])
                wv1 = sb.tile([128, KT * 128], BF16, tag="wb", bufs=2, name="wv1")
                nc.sync.dma_start(wv1[:], wv_d[:, KT * 128:])
                for tt in range(TC // 128):
                    pv = ps.tile([128, KVCOLS], F32, tag="pj", bufs=2, name="pv")
                    for kt in range(KT):
                        lx = xt_tile(c, kt, tt * 128, 128)
                        wvh = wv0 if kt < 16 else wv1
                        nc.tensor.matmul(
                            pv[:], lx,
                            wvh[:, (kt % 16) * KVCOLS:(kt % 16 + 1) * KVCOLS],
                            start=(kt == 0), stop=(kt == KT - 1))
                    vt_idx = (t0 // 128) + tt
                    nc.scalar.copy(
                        vb[:, vt_idx * KVCOLS:(vt_idx + 1) * KVCOLS], pv[:])

            attnb = None
            _WOR_PRE = {}

            def prefetch_wor(ocg):
                """ONE batched DMA for the 8 wo rhs slices of output group
                ocg: tile [128, ft*TC + oc]."""
                w = sb.tile([128, HL * TC], BF16, tag="wor", bufs=2,
                            name="wor")
                nc.sync.dma_start(
                    w.rearrange("p (f c) -> p f c", c=TC),
                    wo_d.rearrange("f p c -> p f c")[:, :, ocg * TC:(ocg + 1) * TC])
                _WOR_PRE[ocg] = w
                return w

            def attention(c):
                """Heads processed in pairs sharing a kv head; adjacent in
                qtb, so one [128, 2*TC] score matmul + one fused exp serve
                both. pa/pd accumulate per head; masks alternate DVE/GpSimd."""
                nonlocal attnb
                nkt = 4 * c + 4
                attnb = sb.tile([128, HL * TC], BF16, tag="attn", bufs=1,
                                name="attnb")
                for p in range(HL // 2):
                    h0, h1 = 2 * p, 2 * p + 1
                    kv = h0 // (HL // KVL)
                    qpair = qtb[:, h0 * TC:(h0 + 2) * TC]
                    pa0 = ps.tile([128, TC], F32, tag="pa", bufs=2, name="pa0")
                    pa1 = ps.tile([128, TC], F32, tag="pa", bufs=2, name="pa1")
                    pd0 = ps.tile([128, TC], F32, tag="pj", bufs=2, name="pd0")
                    pd1 = ps.tile([128, TC], F32, tag="pj", bufs=2, name="pd1")

                    def emit_pa_pd(kt, pt):
                        st, sp = (kt == 0), (kt == nkt - 1)
                        vt = vb[:, kt * KVCOLS + kv * 128:
                                kt * KVCOLS + (kv + 1) * 128]
                        nc.tensor.matmul(pa0[:], vt, pt[:, :TC],
                                         start=st, stop=sp)
                        nc.tensor.matmul(pa1[:], vt, pt[:, TC:],
                                         start=st, stop=sp)
                        nc.tensor.matmul(pd0[:], ones_sb[:], pt[:, :TC],
                                         start=st, stop=sp)
                        nc.tensor.matmul(pd1[:], ones_sb[:], pt[:, TC:],
                                         start=st, stop=sp)

                    prev = None
                    for kt in range(nkt):
                        kts = ktb[:, kv * S + kt * 128:kv * S + (kt + 1) * 128]
                        sps = ps.tile([128, 2 * TC], F32, tag="s", bufs=2,
                                      name="sps")
                        # matmul out must stay within one PSUM bank: write
                        # the fused tile's halves with two 512-wide matmuls
                        # (same stationary K tile), then ONE fused exp.
                        nc.tensor.matmul(sps[:, :TC], kts,
                                         qpair[:, :TC], start=True, stop=True)
                        nc.tensor.matmul(sps[:, TC:], kts,
                                         qpair[:, TC:], start=True, stop=True)
                        if prev is not None:
                            emit_pa_pd(*prev)
                        pt = sb.tile([128, 2 * TC], BF16, tag="pt", bufs=4,
                                     name="pt")
                        nc.scalar.activation(pt[:], sps[:], EXP, scale=SCALE)
                        j = kt - 4 * c
                        if j >= 0:
                            # always DVE: GpSimd must stay free of compute so
                            # a collective's completion wait blocks nothing
                            msk = mask_sb[:, j * 2 * TC:(j + 1) * 2 * TC]
                            nc.vector.tensor_tensor(pt[:], pt[:], msk, MUL)
                        prev = (kt, pt)
                    emit_pa_pd(*prev)
                    rc0 = sb.tile([128, TC], F32, tag="rc", bufs=2, name="rc0")
                    nc.vector.reciprocal(rc0[:], pd0[:])
                    nc.vector.tensor_tensor(
                        attnb[:, h0 * TC:(h0 + 1) * TC], pa0[:], rc0[:], MUL)
                    rc1 = sb.tile([128, TC], F32, tag="rc", bufs=2, name="rc1")
                    nc.vector.reciprocal(rc1[:], pd1[:])
                    nc.vector.tensor_tensor(
                        attnb[:, h1 * TC:(h1 + 1) * TC], pa1[:], rc1[:], MUL)
                    if p == 2:
                        # hoist first o-proj weight loads under attention
                        prefetch_wor(0)

            def outproj(c):
                """Local partial o_proj -> cci[rank, t, oc], then bf16
                ReduceScatter(add) within the 4-core group. The cco->out_t
                DMAs are deferred to the end of the program so no in-order
                DMA queue mid-stream carries a descriptor that has to wait
                for a collective. The last chunk's RS is split in two
                halves (even / odd output groups) so the first RS overlaps
                the second half of the out-projection matmuls."""
                t0 = c * TC
                last = (c == NCHUNK - 1)
                if last:
                    ccis = [dr.tile([TP, TC, OC // 2], BF16, tag="cci",
                                    bufs=2, name=f"cci{h}") for h in range(2)]
                    order = [0, 2, 4, 6, 1, 3, 5, 7]
                else:
                    ccis = [dr.tile([TP, TC, OC], BF16, tag="cci", bufs=2,
                                    name="cci")]
                    order = list(range(8))

                pending_out = []

                def emit_rs(half):
                    """Trigger the RS for one half. The cco->out DMA rides
                    the GpSimd stream (its wait for the RS blocks only
                    GpSimd, which hosts no other work — a sync-engine DMA
                    here would stall ALL DMA dispatch for the collective's
                    latency), and is DEFERRED past any later RS trigger so
                    back-to-back collectives don't serialize on it."""
                    if last:
                        cco = dr.tile([TC, OC // 2], BF16, tag="cco", bufs=5,
                                      name=f"cco{half}")
                        dst = out_d[t0:t0 + TC,
                                    half * (OC // 2):(half + 1) * (OC // 2)]
                    else:
                        cco = dr.tile([TC, OC], BF16, tag="cco", bufs=5,
                                      name="cco")
                        dst = out_d[t0:t0 + TC, :]
                    nc.gpsimd.collective_compute(
                        "ReduceScatter", ADD,
                        replica_groups=GROUPS, ins=[ccis[half][:]],
                        outs=[cco[:]])
                    pending_out.append((dst, cco))

                def flush_out():
                    for dst, cco in pending_out:
                        nc.gpsimd.dma_start(dst, cco[:])
                    pending_out.clear()

                for i, ocg in enumerate(order):
                    wor = _WOR_PRE.pop(ocg, None)
                    if wor is None:
                        wor = prefetch_wor(ocg)
                    if i + 1 < 8:
                        prefetch_wor(order[i + 1])
                    if last:
                        cci, ocs = ccis[ocg % 2], 0
                    else:
                        cci, ocs = ccis[0], (ocg % 2) * TC
                    ot = sb.tile([128, 4 * TC], BF16, tag="ot", bufs=2,
                                 name="ot")
                    for tt in range(TC // 128):
                        po = ps.tile([128, TC], F32, tag="s", bufs=2, name="po")
                        for ft in range(HL):
                            nc.tensor.matmul(
                                po[:],
                                attnb[:, ft * TC + tt * 128:ft * TC + (tt + 1) * 128],
                                wor[:, ft * TC:(ft + 1) * TC],
                                start=(ft == 0), stop=(ft == HL - 1))
                        nc.scalar.copy(ot[:, tt * TC:(tt + 1) * TC], po[:])
                    nc.sync.dma_start(
                        cci[ocg // 2].rearrange(
                            "(tt p) oc -> p tt oc", p=128)[:, :, ocs:ocs + TC],
                        ot.rearrange("p (tt oc) -> p tt oc", oc=TC))
                    if last and i == 3:
                        emit_rs(0)
                if last:
                    emit_rs(1)
                else:
                    emit_rs(0)
                flush_out()

            # ---- chunk loop ----
            load_xt(0)
            for c in range(NCHUNK):
                proj(c)
                attention(c)
                outproj(c)

    nc.compile()
    return nc


def _get_nc():
    if "nc" not in _BUILT:
        _BUILT["nc"] = _build()
    return _BUILT["nc"]


def kernel(hidden_states, cos, sin, wq, wk, wv, wo):
    global LAST_RESULT
    nc = _get_nc()

    hidden_states = np.asarray(hidden_states, dtype=np.float32)
    cos = np.asarray(cos, dtype=np.float32)
    sin = np.asarray(sin, dtype=np.float32)
    wq = np.asarray(wq, dtype=np.float32)
    wk = np.asarray(wk, dtype=np.float32)
    wv = np.asarray(wv, dtype=np.float32)
    wo = np.asarray(wo, dtype=np.float32)

    # host-side shard prep (bf16, panel-major so every DMA is contiguous)
    xts = [np.ascontiguousarray(hidden_states[b].T).astype(B16NP)
           for b in range(B)]
    cts = [np.ascontiguousarray(cos[b].T).astype(B16NP) for b in range(B)]
    sin_eff = []
    for b in range(B):
        se = np.ascontiguousarray(sin[b].T).copy()
        se[0:64, :] *= -1.0
        sin_eff.append(se.astype(B16NP))

    # fused pair masks: each diagonal mask duplicated for the head pair
    masks = np.zeros((HD, 8 * TC), dtype=B16NP)
    for j in range(4):
        m = ((np.arange(HD)[:, None] + 128 * j)
             <= np.arange(TC)[None, :]).astype(B16NP)
        masks[:, j * 2 * TC:j * 2 * TC + TC] = m
        masks[:, j * 2 * TC + TC:(j + 1) * 2 * TC] = m
    onesb = np.ones((128, 128), dtype=B16NP)

    in_maps = []
    for core in range(N_CORES):
        b, r = core // TP, core % TP
        wq_s = wq[:, r * QCOLS:(r + 1) * QCOLS]
        wk_s = wk[:, r * KVCOLS:(r + 1) * KVCOLS]
        wv_s = wv[:, r * KVCOLS:(r + 1) * KVCOLS]
        wo_s = wo[r * QCOLS:(r + 1) * QCOLS, :]
        wqp = np.ascontiguousarray(
            wq_s.reshape(KT, 128, HL, 128).transpose(2, 1, 0, 3)
            .reshape(HL, 128, KT * 128)).astype(B16NP)
        wkp = np.ascontiguousarray(
            wk_s.reshape(KT, 128, KVL, 128).transpose(2, 1, 0, 3)
            .reshape(KVL, 128, KT * 128)).astype(B16NP)
        wvp = np.ascontiguousarray(
            wv_s.reshape(KT, 128, KVCOLS).transpose(1, 0, 2)
            .reshape(128, KT * KVCOLS)).astype(B16NP)
        wop = np.ascontiguousarray(
            wo_s.reshape(HL, 128, H)).astype(B16NP)
        in_maps.append({
            "xt": xts[b],
            "cos_t": cts[b],
            "sin_t": sin_eff[b],
            "wqp": wqp,
            "wkp": wkp,
            "wvp": wvp,
            "wop": wop,
            "masks": masks,
            "onesb": onesb,
        })

    res = run_bass_kernel_spmd(nc, in_maps, core_ids=list(range(N_CORES)))
    LAST_RESULT = res

    out = np.empty((B, S, H), dtype=np.float32)
    for core in range(N_CORES):
        b, r = core // TP, core % TP
        out[b, :, r * OC:(r + 1) * OC] = res.results[core]["out_t"].astype(
            np.float32)
    return out


# revision 23
# speedup vs baseline: 1.1344x; 1.0006x over previous
"""Llama GQA attention (B=2, S=2048, H=4096, 32 q heads / 8 kv heads, HD=128)
on 8 Trainium2 NeuronCores.

Sharding: DP=2 over batch x TP=4 over heads.
  core c: batch b = c // 4, tp rank r = c % 4
  - owns q heads [8r, 8r+8), kv heads [2r, 2r+2)
  - computes attention for its heads over its batch
  - o_proj: LOCAL partial over its 1024 attn features for ALL 4096 out
    cols (wo sharded by ROWS), then bf16 ReduceScatter(add) within each
    4-core batch group -> each core holds out cols [1024r, 1024(r+1)).

All inputs are pre-cast to bf16 on the host (weights in panel-major
layouts so every DMA is contiguous), so the device pipeline has zero
dtype-conversion work and no DRAM scratch round-trips.

On-chip layout is fully "transposed" ([feature, token]):
  QT/KT: [d, t] (weight tiles stationary, X^T moving)
  V:     [t, d] (X^T tiles stationary, wv moving)
  S^T[k, q] = (KT tile).T @ QT          (contraction d on partitions)
  P^T = exp(scale * S^T)                (ScalarE, fp32 PSUM -> bf16 SBUF)
  attn^T[d, q] += (V tile).T @ P^T      (contraction k-tokens on partitions)
  out[t, oc]  += (attnb tile).T @ wo    (contraction d on partitions)
Causal masking: only lower-triangle k-tiles are computed; diagonal
128x512 tiles use one of 4 static 0/1 masks (multiplied into P^T on DVE).
Softmax skips max-subtraction (scores are O(7), exp fits fp32).

Denominators: P^T tiles are accumulated on the Pool engine into a
per-head running sum (bf16), then ONE ones-matmul per (head, chunk)
produces the denominator — instead of one matmul per k-tile.

Attention processes heads in PAIRS sharing the same kv head, software-
pipelined so the exp->mask->pa chain of one head hides under the other
head's matmuls:
  per kt: [score(h0), score(h1), pa(h0, kt-1), pa(h1, kt-1)]
X^T for chunk c+1 is DMA'd (pure bf16 copy, no engine work) into a
2-slot rotation at the start of proj(c).
"""

import os
import sys

for _p in ("/opt/trn_rl_repo",):
    if _p not in sys.path:
        sys.path.append(_p)

import numpy as np
import ml_dtypes

import concourse.bacc as bacc
import concourse.mybir as mybir
import concourse.tile as tile
from concourse.bass_utils import run_bass_kernel_spmd

F32 = mybir.dt.float32
BF16 = mybir.dt.bfloat16
B16NP = ml_dtypes.bfloat16

B, S, H = 2, 2048, 4096
NH, NKV, HD = 32, 8, 128
N_CORES = 8
TP = 4
GROUPS = [[0, 1, 2, 3], [4, 5, 6, 7]]

HL = NH // TP          # 8 local q heads
KVL = NKV // TP        # 2 local kv heads
QCOLS = HL * HD        # 1024 local q cols
KVCOLS = KVL * HD      # 256 local kv cols
OC = H // TP           # 1024 out cols per core after ReduceScatter

TC = 512               # token chunk (= one attention q-block)
NCHUNK = S // TC       # 4
KT = H // 128          # 32 contraction tiles for the projections
SCALE = float(HD ** -0.5)

LAST_RESULT = None
_BUILT = {}

EXP = mybir.ActivationFunctionType.Exp
MUL = mybir.AluOpType.mult
ADD = mybir.AluOpType.add


def _build():
    nc = bacc.Bacc("TRN2", debug=False, num_devices=N_CORES)

    xt_d = nc.dram_tensor("xt", [H, S], BF16, kind="ExternalInput").ap()
    cos_d = nc.dram_tensor("cos_t", [HD, S], BF16, kind="ExternalInput").ap()
    sin_d = nc.dram_tensor("sin_t", [HD, S], BF16, kind="ExternalInput").ap()
    wq_d = nc.dram_tensor("wqp", [HL, 128, KT * 128], BF16, kind="ExternalInput").ap()
    wk_d = nc.dram_tensor("wkp", [KVL, 128, KT * 128], BF16, kind="ExternalInput").ap()
    wv_d = nc.dram_tensor("wvp", [128, KT * KVCOLS], BF16, kind="ExternalInput").ap()
    wo_d = nc.dram_tensor("wop", [HL, 128, H], BF16, kind="ExternalInput").ap()
    mask_d = nc.dram_tensor("masks", [HD, 8 * TC], BF16, kind="ExternalInput").ap()
    ones_d = nc.dram_tensor("onesb", [128, 128], BF16, kind="ExternalInput").ap()
    out_d = nc.dram_tensor("out_t", [S, OC], BF16, kind="ExternalOutput").ap()

    with tile.TileContext(nc) as tc:
        with tc.tile_pool(name="sb", bufs=1) as sb, \
             tc.tile_pool(name="ps", bufs=1, space="PSUM") as ps, \
             tc.tile_pool(name="dr", bufs=1, space="DRAM") as dr:

            # ---- persistent tiles ----
            cos_sb = sb.tile([HD, S], BF16)
            sin_sb = sb.tile([HD, S], BF16)
            mask_sb = sb.tile([HD, 8 * TC], BF16)
            ones_sb = sb.tile([128, 128], BF16)
            ktb = sb.tile([128, KVL * S], BF16)             # roped K^T
            vb = sb.tile([128, (S // 128) * KVCOLS], BF16)  # V, [t, tt*256 + d]
            # X^T bf16, 2-chunk rotation: [128, kt*(2*TC) + slot*TC + t]
            xtb = sb.tile([128, KT * 2 * TC], BF16)
            nc.sync.dma_start(cos_sb[:], cos_d[:])
            nc.sync.dma_start(sin_sb[:], sin_d[:])
            nc.sync.dma_start(mask_sb[:], mask_d[:])
            nc.sync.dma_start(ones_sb[:], ones_d[:])

            def xt_slot(c):
                return c % 2

            def load_xt(c):
                """Four batched DMAs (8 kt-tiles each) bringing chunk c of
                X^T into its rotation slot. (Each DMA instruction costs
                ~0.7us of serial dispatch on the Sync sequencer — batch;
                but keep kt-granularity coarse enough for the first
                matmuls to start before the whole chunk lands.)"""
                sl = xt_slot(c)
                dst = xtb.rearrange("p (kt s t) -> p kt s t", s=2, t=TC)[:, :, sl, :]
                src = xt_d[:, c * TC:(c + 1) * TC].rearrange(
                    "(kt p) t -> p kt t", p=128)
                for g in range(4):
                    nc.sync.dma_start(dst[:, g * 8:(g + 1) * 8, :],
                                      src[:, g * 8:(g + 1) * 8, :])

            def xt_tile(c, kt, lo=0, width=TC):
                sl = xt_slot(c)
                base = kt * 2 * TC + sl * TC + lo
                return xtb[:, base:base + width]

            def rope(dst, pq, t0):
                """dst (bf16 [128, TC]) = rope of pq (fp32 PSUM [128, TC])."""
                qf = sb.tile([128, TC], F32, tag="qf", bufs=2)
                nc.scalar.copy(qf[:], pq[:])
                qs = sb.tile([128, TC], F32, tag="qs", bufs=2)
                # half-swap DMAs on the Scalar engine's own DGE: they are
                # ready the moment the qf copy (same engine) retires, and
                # they stay off the Sync dispatch stream, where their wait
                # would delay later weight-panel dispatches.
                nc.scalar.dma_start(qs[0:64, :], qf[64:128, :])
                nc.scalar.dma_start(qs[64:128, :], qf[0:64, :])
                nc.vector.tensor_tensor(qf[:], qf[:], cos_sb[:, t0:t0 + TC], MUL)
                nc.vector.tensor_tensor(qs[:], qs[:], sin_sb[:, t0:t0 + TC], MUL)
                nc.vector.tensor_tensor(dst, qf[:], qs[:], ADD)

            qtb = None

            def proj(c):
                nonlocal qtb
                t0 = c * TC
                qtb = sb.tile([128, HL * TC], BF16, tag="qt", bufs=1, name="qtb")
                for h in range(HL):
                    wb = sb.tile([128, KT * 128], BF16, tag="wb", bufs=2, name="wb")
                    nc.sync.dma_start(wb[:], wq_d[h])
                    pq = ps.tile([128, TC], F32, tag="pj", bufs=2, name="pq")
                    for kt in range(KT):
                        nc.tensor.matmul(
                            pq[:], wb[:, kt * 128:(kt + 1) * 128],
                            xt_tile(c, kt),
                            start=(kt == 0), stop=(kt == KT - 1))
                    rope(qtb[:, h * TC:(h + 1) * TC], pq, t0)
                    if h == 1 and c + 1 < NCHUNK:
                        # next chunk's X^T, after the first panels are in
                        # flight so it doesn't delay them in dispatch order
                        load_xt(c + 1)
                for kv in range(KVL):
                    wb = sb.tile([128, KT * 128], BF16, tag="wb", bufs=2, name="wbk")
                    nc.sync.dma_start(wb[:], wk_d[kv])
                    pk = ps.tile([128, TC], F32, tag="pj", bufs=2, name="pk")
                    for kt in range(KT):
                        nc.tensor.matmul(
                            pk[:], wb[:, kt * 128:(kt + 1) * 128],
                            xt_tile(c, kt),
                            start=(kt == 0), stop=(kt == KT - 1))
                    rope(ktb[:, kv * S + t0:kv * S + t0 + TC], pk, t0)
                # V: lhsT = X^T tiles (stationary), rhs = wv (both kv heads at
                # once, 256-wide) -> V[t, c] accumulated over kt.
                wv0 = sb.tile([128, KT * 128], BF16, tag="wb", bufs=2, name="wv0")
                nc.sync.dma_start(wv0[:], wv_d[:, :KT * 128])
                wv1 = sb.tile([128, KT * 128], BF16, tag="wb", bufs=2, name="wv1")
                nc.sync.dma_start(wv1[:], wv_d[:, KT * 128:])
                for tt in range(TC // 128):
                    pv = ps.tile([128, KVCOLS], F32, tag="pj", bufs=2, name="pv")
                    for kt in range(KT):
                        lx = xt_tile(c, kt, tt * 128, 128)
                        wvh = wv0 if kt < 16 else wv1
                        nc.tensor.matmul(
                            pv[:], lx,
                            wvh[:, (kt % 16) * KVCOLS:(kt % 16 + 1) * KVCOLS],
                            start=(kt == 0), stop=(kt == KT - 1))
                    vt_idx = (t0 // 128) + tt
                    nc.scalar.copy(
                        vb[:, vt_idx * KVCOLS:(vt_idx + 1) * KVCOLS], pv[:])

            attnb = None
            _WOR_PRE = {}

            def prefetch_wor(ocg):
                """ONE batched DMA for the 8 wo rhs slices of output group
                ocg: tile [128, ft*TC + oc]."""
                w = sb.tile([128, HL * TC], BF16, tag="wor", bufs=2,
                            name="wor")
                nc.sync.dma_start(
                    w.rearrange("p (f c) -> p f c", c=TC),
                    wo_d.rearrange("f p c -> p f c")[:, :, ocg * TC:(ocg + 1) * TC])
                _WOR_PRE[ocg] = w
                return w

            def attention(c):
                """Heads processed in pairs sharing a kv head; adjacent in
                qtb, so one [128, 2*TC] score matmul + one fused exp serve
                both. pa/pd accumulate per head; masks alternate DVE/GpSimd."""
                nonlocal attnb
                nkt = 4 * c + 4
                attnb = sb.tile([128, HL * TC], BF16, tag="attn", bufs=1,
                                name="attnb")
                for p in range(HL // 2):
                    h0, h1 = 2 * p, 2 * p + 1
                    kv = h0 // (HL // KVL)
                    qpair = qtb[:, h0 * TC:(h0 + 2) * TC]
                    pa0 = ps.tile([128, TC], F32, tag="pa", bufs=2, name="pa0")
                    pa1 = ps.tile([128, TC], F32, tag="pa", bufs=2, name="pa1")
                    pd0 = ps.tile([128, TC], F32, tag="pj", bufs=2, name="pd0")
                    pd1 = ps.tile([128, TC], F32, tag="pj", bufs=2, name="pd1")

                    def emit_pa_pd(kt, pt):
                        st, sp = (kt == 0), (kt == nkt - 1)
                        vt = vb[:, kt * KVCOLS + kv * 128:
                                kt * KVCOLS + (kv + 1) * 128]
                        nc.tensor.matmul(pa0[:], vt, pt[:, :TC],
                                         start=st, stop=sp)
                        nc.tensor.matmul(pa1[:], vt, pt[:, TC:],
                                         start=st, stop=sp)
                        nc.tensor.matmul(pd0[:], ones_sb[:], pt[:, :TC],
                                         start=st, stop=sp)
                        nc.tensor.matmul(pd1[:], ones_sb[:], pt[:, TC:],
                                         start=st, stop=sp)

                    pending = []
                    for kt in range(nkt):
                        kts = ktb[:, kv * S + kt * 128:kv * S + (kt + 1) * 128]
                        sps = ps.tile([128, 2 * TC], F32, tag="s", bufs=2,
                                      name="sps")
                        # matmul out must stay within one PSUM bank: write
                        # the fused tile's halves with two 512-wide matmuls
                        # (same stationary K tile), then ONE fused exp.
                        nc.tensor.matmul(sps[:, :TC], kts,
                                         qpair[:, :TC], start=True, stop=True)
                        nc.tensor.matmul(sps[:, TC:], kts,
                                         qpair[:, TC:], start=True, stop=True)
                        # pa/pd run TWO k-tiles behind the scores: the extra
                        # slack hides exp latency AND the previous pair's
                        # normalize chain (which frees the pa psum slots).
                        if len(pending) >= 2:
                            emit_pa_pd(*pending.pop(0))
                        pt = sb.tile([128, 2 * TC], BF16, tag="pt", bufs=4,
                                     name="pt")
                        nc.scalar.activation(pt[:], sps[:], EXP, scale=SCALE)
                        j = kt - 4 * c
                        if j >= 0:
                            # always DVE: GpSimd must stay free of compute so
                            # a collective's completion wait blocks nothing
                            msk = mask_sb[:, j * 2 * TC:(j + 1) * 2 * TC]
                            nc.vector.tensor_tensor(pt[:], pt[:], msk, MUL)
                        pending.append((kt, pt))
                    while pending:
                        emit_pa_pd(*pending.pop(0))
                    rc0 = sb.tile([128, TC], F32, tag="rc", bufs=2, name="rc0")
                    nc.vector.reciprocal(rc0[:], pd0[:])
                    nc.vector.tensor_tensor(
                        attnb[:, h0 * TC:(h0 + 1) * TC], pa0[:], rc0[:], MUL)
                    rc1 = sb.tile([128, TC], F32, tag="rc", bufs=2, name="rc1")
                    nc.vector.reciprocal(rc1[:], pd1[:])
                    nc.vector.tensor_tensor(
                        attnb[:, h1 * TC:(h1 + 1) * TC], pa1[:], rc1[:], MUL)
                    if p == 2:
                        # hoist first o-proj weight loads under attention
                        prefetch_wor(0)

            def outproj(c):
                """Local partial o_proj -> cci[rank, t, oc], then bf16
                ReduceScatter(add) within the 4-core group. The cco->out_t
                DMAs are deferred to the end of the program so no in-order
                DMA queue mid-stream carries a descriptor that has to wait
                for a collective. The last chunk's RS is split in two
                halves (even / odd output groups) so the first RS overlaps
                the second half of the out-projection matmuls."""
                t0 = c * TC
                ccis = [dr.tile([TP, TC, OC // 2], BF16, tag="cci",
                                bufs=4, name=f"cci{h}") for h in range(2)]
                order = [0, 2, 4, 6, 1, 3, 5, 7]

                pending_out = []

                def emit_rs(half):
                    """Trigger the RS for one half of the output columns.
                    The cco->out DMA rides the GpSimd stream (its wait for
                    the RS blocks only GpSimd, which hosts no other work —
                    a sync-engine DMA here would stall ALL DMA dispatch for
                    the collective's latency), and is DEFERRED past any
                    later RS trigger so back-to-back collectives don't
                    serialize on it."""
                    cco = dr.tile([TC, OC // 2], BF16, tag="cco", bufs=4,
                                  name=f"cco{half}")
                    dst = out_d[t0:t0 + TC,
                                half * (OC // 2):(half + 1) * (OC // 2)]
                    nc.gpsimd.collective_compute(
                        "ReduceScatter", ADD,
                        replica_groups=GROUPS, ins=[ccis[half][:]],
                        outs=[cco[:]])
                    pending_out.append((dst, cco))

                for i, ocg in enumerate(order):
                    wor = _WOR_PRE.pop(ocg, None)
                    if wor is None:
                        wor = prefetch_wor(ocg)
                    if i + 1 < 8:
                        prefetch_wor(order[i + 1])
                    cci = ccis[ocg % 2]
                    ot = sb.tile([128, 4 * TC], BF16, tag="ot", bufs=2,
                                 name="ot")
                    for tt in range(TC // 128):
                        po = ps.tile([128, TC], F32, tag="s", bufs=2, name="po")
                        for ft in range(HL):
                            nc.tensor.matmul(
                                po[:],
                                attnb[:, ft * TC + tt * 128:ft * TC + (tt + 1) * 128],
                                wor[:, ft * TC:(ft + 1) * TC],
                                start=(ft == 0), stop=(ft == HL - 1))
                        nc.scalar.copy(ot[:, tt * TC:(tt + 1) * TC], po[:])
                    nc.sync.dma_start(
                        cci[ocg // 2].rearrange("(tt p) oc -> p tt oc", p=128),
                        ot.rearrange("p (tt oc) -> p tt oc", oc=TC))
                    if i == 3:
                        emit_rs(0)
                emit_rs(1)
                for dst, cco in pending_out:
                    nc.gpsimd.dma_start(dst, cco[:])

            # ---- chunk loop ----
            load_xt(0)
            for c in range(NCHUNK):
                proj(c)
                attention(c)
                outproj(c)

    nc.compile()
    return nc


def _get_nc():
    if "nc" not in _BUILT:
        _BUILT["nc"] = _build()
    return _BUILT["nc"]


def kernel(hidden_states, cos, sin, wq, wk, wv, wo):
    global LAST_RESULT
    nc = _get_nc()

    hidden_states = np.asarray(hidden_states, dtype=np.float32)
    cos = np.asarray(cos, dtype=np.float32)
    sin = np.asarray(sin, dtype=np.float32)
    wq = np.asarray(wq, dtype=np.float32)
    wk = np.asarray(wk, dtype=np.float32)
    wv = np.asarray(wv, dtype=np.float32)
    wo = np.asarray(wo, dtype=np.float32)

    # host-side shard prep (bf16, panel-major so every DMA is contiguous)
    xts = [np.ascontiguousarray(hidden_states[b].T).astype(B16NP)
           for b in range(B)]
    cts = [np.ascontiguousarray(cos[b].T).astype(B16NP) for b in range(B)]
    sin_eff = []
    for b in range(B):
        se = np.ascontiguousarray(sin[b].T).copy()
        se[0:64, :] *= -1.0
        sin_eff.append(se.astype(B16NP))

    # fused pair masks: each diagonal mask duplicated for the head pair
    masks = np.zeros((HD, 8 * TC), dtype=B16NP)
    for j in range(4):
        m = ((np.arange(HD)[:, None] + 128 * j)
             <= np.arange(TC)[None, :]).astype(B16NP)
        masks[:, j * 2 * TC:j * 2 * TC + TC] = m
        masks[:, j * 2 * TC + TC:(j + 1) * 2 * TC] = m
    onesb = np.ones((128, 128), dtype=B16NP)

    in_maps = []
    for core in range(N_CORES):
        b, r = core // TP, core % TP
        wq_s = wq[:, r * QCOLS:(r + 1) * QCOLS]
        wk_s = wk[:, r * KVCOLS:(r + 1) * KVCOLS]
        wv_s = wv[:, r * KVCOLS:(r + 1) * KVCOLS]
        wo_s = wo[r * QCOLS:(r + 1) * QCOLS, :]
        wqp = np.ascontiguousarray(
            wq_s.reshape(KT, 128, HL, 128).transpose(2, 1, 0, 3)
            .reshape(HL, 128, KT * 128)).astype(B16NP)
        wkp = np.ascontiguousarray(
            wk_s.reshape(KT, 128, KVL, 128).transpose(2, 1, 0, 3)
            .reshape(KVL, 128, KT * 128)).astype(B16NP)
        wvp = np.ascontiguousarray(
            wv_s.reshape(KT, 128, KVCOLS).transpose(1, 0, 2)
            .reshape(128, KT * KVCOLS)).astype(B16NP)
        wop = np.ascontiguousarray(
            wo_s.reshape(HL, 128, H)).astype(B16NP)
        in_maps.append({
            "xt": xts[b],
            "cos_t": cts[b],
            "sin_t": sin_eff[b],
            "wqp": wqp,
            "wkp": wkp,
            "wvp": wvp,
            "wop": wop,
            "masks": masks,
            "onesb": onesb,
        })

    res = run_bass_kernel_spmd(nc, in_maps, core_ids=list(range(N_CORES)))
    LAST_RESULT = res

    out = np.empty((B, S, H), dtype=np.float32)
    for core in range(N_CORES):
        b, r = core // TP, core % TP
        out[b, :, r * OC:(r + 1) * OC] = res.results[core]["out_t"].astype(
            np.float32)
    return out


# revision 24
# speedup vs baseline: 1.2909x; 1.1380x over previous
"""Llama GQA attention (B=2, S=2048, H=4096, 32 q heads / 8 kv heads, HD=128)
on 8 Trainium2 NeuronCores.

Sharding: DP=2 over batch x TP=4 over heads.
  core c: batch b = c // 4, tp rank r = c % 4
  - owns q heads [8r, 8r+8), kv heads [2r, 2r+2)
  - computes attention for its heads over its batch
  - o_proj: LOCAL partial over its 1024 attn features for ALL 4096 out
    cols (wo sharded by ROWS), then bf16 ReduceScatter(add) within each
    4-core batch group -> each core holds out cols [1024r, 1024(r+1)).

All inputs are pre-cast to bf16 on the host (weights in panel-major
layouts so every DMA is contiguous), so the device pipeline has zero
dtype-conversion work and no DRAM scratch round-trips.

On-chip layout is fully "transposed" ([feature, token]):
  QT/KT: [d, t] (weight tiles stationary, X^T moving)
  V:     [t, d] (X^T tiles stationary, wv moving)
  S^T[k, q] = (KT tile).T @ QT          (contraction d on partitions)
  P^T = exp(scale * S^T)                (ScalarE, fp32 PSUM -> bf16 SBUF)
  attn^T[d, q] += (V tile).T @ P^T      (contraction k-tokens on partitions)
  out[t, oc]  += (attnb tile).T @ wo    (contraction d on partitions)
Causal masking: only lower-triangle k-tiles are computed; diagonal
128x512 tiles use one of 4 static 0/1 masks (multiplied into P^T on DVE).
Softmax skips max-subtraction (scores are O(7), exp fits fp32).

Denominators: P^T tiles are accumulated on the Pool engine into a
per-head running sum (bf16), then ONE ones-matmul per (head, chunk)
produces the denominator — instead of one matmul per k-tile.

Attention processes heads in PAIRS sharing the same kv head, software-
pipelined so the exp->mask->pa chain of one head hides under the other
head's matmuls:
  per kt: [score(h0), score(h1), pa(h0, kt-1), pa(h1, kt-1)]
X^T for chunk c+1 is DMA'd (pure bf16 copy, no engine work) into a
2-slot rotation at the start of proj(c).
"""

import os
import sys

for _p in ("/opt/trn_rl_repo",):
    if _p not in sys.path:
        sys.path.append(_p)

import numpy as np
import ml_dtypes

import concourse.bacc as bacc
import concourse.mybir as mybir
import concourse.tile as tile
from concourse.bass_utils import run_bass_kernel_spmd

F32 = mybir.dt.float32
BF16 = mybir.dt.bfloat16
B16NP = ml_dtypes.bfloat16

B, S, H = 2, 2048, 4096
NH, NKV, HD = 32, 8, 128
N_CORES = 8
TP = 4
GROUPS = [[0, 1, 2, 3], [4, 5, 6, 7]]

HL = NH // TP          # 8 local q heads
KVL = NKV // TP        # 2 local kv heads
QCOLS = HL * HD        # 1024 local q cols
KVCOLS = KVL * HD      # 256 local kv cols
OC = H // TP           # 1024 out cols per core after ReduceScatter

TC = 512               # token chunk (= one attention q-block)
NCHUNK = S // TC       # 4
KT = H // 128          # 32 contraction tiles for the projections
SCALE = float(HD ** -0.5)

LAST_RESULT = None
_BUILT = {}

EXP = mybir.ActivationFunctionType.Exp
MUL = mybir.AluOpType.mult
ADD = mybir.AluOpType.add


def _build():
    nc = bacc.Bacc("TRN2", debug=False, num_devices=N_CORES)

    xt_d = nc.dram_tensor("xt", [H, S], BF16, kind="ExternalInput").ap()
    cos_d = nc.dram_tensor("cos_t", [HD, S], BF16, kind="ExternalInput").ap()
    sin_d = nc.dram_tensor("sin_t", [HD, S], BF16, kind="ExternalInput").ap()
    wq_d = nc.dram_tensor("wqp", [HL, 128, KT * 128], BF16, kind="ExternalInput").ap()
    wk_d = nc.dram_tensor("wkp", [KVL, 128, KT * 128], BF16, kind="ExternalInput").ap()
    wv_d = nc.dram_tensor("wvp", [128, KT * KVCOLS], BF16, kind="ExternalInput").ap()
    wo_d = nc.dram_tensor("wop", [HL, 128, H], BF16, kind="ExternalInput").ap()
    mask_d = nc.dram_tensor("masks", [HD, 8 * TC], BF16, kind="ExternalInput").ap()
    ones_d = nc.dram_tensor("onesb", [128, 128], BF16, kind="ExternalInput").ap()
    out_d = nc.dram_tensor("out_t", [S, OC], BF16, kind="ExternalOutput").ap()

    with tile.TileContext(nc) as tc:
        with tc.tile_pool(name="sb", bufs=1) as sb, \
             tc.tile_pool(name="ps", bufs=1, space="PSUM") as ps, \
             tc.tile_pool(name="dr", bufs=1, space="DRAM") as dr:

            # ---- persistent tiles ----
            cos_sb = sb.tile([HD, S], BF16)
            sin_sb = sb.tile([HD, S], BF16)
            mask_sb = sb.tile([HD, 8 * TC], BF16)
            ones_sb = sb.tile([128, 128], BF16)
            ktb = sb.tile([128, KVL * S], BF16)             # roped K^T
            vb = sb.tile([128, (S // 128) * KVCOLS], BF16)  # V, [t, tt*256 + d]
            # X^T bf16, 2-chunk rotation: [128, kt*(2*TC) + slot*TC + t]
            xtb = sb.tile([128, KT * 2 * TC], BF16)
            nc.sync.dma_start(cos_sb[:], cos_d[:])
            nc.sync.dma_start(sin_sb[:], sin_d[:])
            nc.sync.dma_start(mask_sb[:], mask_d[:])
            nc.sync.dma_start(ones_sb[:], ones_d[:])

            def xt_slot(c):
                return c % 2

            def load_xt(c):
                """Four batched DMAs (8 kt-tiles each) bringing chunk c of
                X^T into its rotation slot. (Each DMA instruction costs
                ~0.7us of serial dispatch on the Sync sequencer — batch;
                but keep kt-granularity coarse enough for the first
                matmuls to start before the whole chunk lands.)"""
                sl = xt_slot(c)
                dst = xtb.rearrange("p (kt s t) -> p kt s t", s=2, t=TC)[:, :, sl, :]
                src = xt_d[:, c * TC:(c + 1) * TC].rearrange(
                    "(kt p) t -> p kt t", p=128)
                for g in range(4):
                    nc.sync.dma_start(dst[:, g * 8:(g + 1) * 8, :],
                                      src[:, g * 8:(g + 1) * 8, :])

            def xt_tile(c, kt, lo=0, width=TC):
                sl = xt_slot(c)
                base = kt * 2 * TC + sl * TC + lo
                return xtb[:, base:base + width]

            def rope(dst, pq, t0):
                """dst (bf16 [128, TC]) = rope of pq (fp32 PSUM [128, TC])."""
                qf = sb.tile([128, TC], F32, tag="qf", bufs=2)
                nc.scalar.copy(qf[:], pq[:])
                qs = sb.tile([128, TC], F32, tag="qs", bufs=2)
                # half-swap DMAs on the Scalar engine's own DGE: they are
                # ready the moment the qf copy (same engine) retires, and
                # they stay off the Sync dispatch stream, where their wait
                # would delay later weight-panel dispatches.
                nc.scalar.dma_start(qs[0:64, :], qf[64:128, :])
                nc.scalar.dma_start(qs[64:128, :], qf[0:64, :])
                nc.vector.tensor_tensor(qf[:], qf[:], cos_sb[:, t0:t0 + TC], MUL)
                nc.vector.tensor_tensor(qs[:], qs[:], sin_sb[:, t0:t0 + TC], MUL)
                nc.vector.tensor_tensor(dst, qf[:], qs[:], ADD)

            qtb = None

            def proj(c):
                nonlocal qtb
                t0 = c * TC
                qtb = sb.tile([128, HL * TC], BF16, tag="qt", bufs=1, name="qtb")
                for h in range(HL):
                    wb = sb.tile([128, KT * 128], BF16, tag="wb", bufs=3, name="wb")
                    nc.sync.dma_start(wb[:], wq_d[h])
                    pq = ps.tile([128, TC], F32, tag="pj", bufs=2, name="pq")
                    for kt in range(KT):
                        nc.tensor.matmul(
                            pq[:], wb[:, kt * 128:(kt + 1) * 128],
                            xt_tile(c, kt),
                            start=(kt == 0), stop=(kt == KT - 1))
                    rope(qtb[:, h * TC:(h + 1) * TC], pq, t0)
                    if h == 1 and c + 1 < NCHUNK:
                        # next chunk's X^T, after the first panels are in
                        # flight so it doesn't delay them in dispatch order
                        load_xt(c + 1)
                for kv in range(KVL):
                    wb = sb.tile([128, KT * 128], BF16, tag="wb", bufs=3, name="wbk")
                    nc.sync.dma_start(wb[:], wk_d[kv])
                    pk = ps.tile([128, TC], F32, tag="pj", bufs=2, name="pk")
                    for kt in range(KT):
                        nc.tensor.matmul(
                            pk[:], wb[:, kt * 128:(kt + 1) * 128],
                            xt_tile(c, kt),
                            start=(kt == 0), stop=(kt == KT - 1))
                    rope(ktb[:, kv * S + t0:kv * S + t0 + TC], pk, t0)
                # V: lhsT = X^T tiles (stationary), rhs = wv (both kv heads at
                # once, 256-wide) -> V[t, c] accumulated over kt.
                wv0 = sb.tile([128, KT * 128], BF16, tag="wb", bufs=3, name="wv0")
                nc.sync.dma_start(wv0[:], wv_d[:, :KT * 128])
                wv1 = sb.tile([128, KT * 128], BF16, tag="wb", bufs=3, name="wv1")
                nc.sync.dma_start(wv1[:], wv_d[:, KT * 128:])
                for tt in range(TC // 128):
                    pv = ps.tile([128, KVCOLS], F32, tag="pj", bufs=2, name="pv")
                    for kt in range(KT):
                        lx = xt_tile(c, kt, tt * 128, 128)
                        wvh = wv0 if kt < 16 else wv1
                        nc.tensor.matmul(
                            pv[:], lx,
                            wvh[:, (kt % 16) * KVCOLS:(kt % 16 + 1) * KVCOLS],
                            start=(kt == 0), stop=(kt == KT - 1))
                    vt_idx = (t0 // 128) + tt
                    nc.scalar.copy(
                        vb[:, vt_idx * KVCOLS:(vt_idx + 1) * KVCOLS], pv[:])

            attnb = None
            _WOR_PRE = {}

            def prefetch_wor(ocg):
                """ONE batched DMA for the 8 wo rhs slices of output group
                ocg: tile [128, ft*TC + oc]."""
                w = sb.tile([128, HL * TC], BF16, tag="wor", bufs=2,
                            name="wor")
                nc.sync.dma_start(
                    w.rearrange("p (f c) -> p f c", c=TC),
                    wo_d.rearrange("f p c -> p f c")[:, :, ocg * TC:(ocg + 1) * TC])
                _WOR_PRE[ocg] = w
                return w

            def attention(c):
                """Heads processed in pairs sharing a kv head; adjacent in
                qtb, so one [128, 2*TC] score matmul + one fused exp serve
                both. pa/pd accumulate per head; masks alternate DVE/GpSimd."""
                nonlocal attnb
                nkt = 4 * c + 4
                attnb = sb.tile([128, HL * TC], BF16, tag="attn", bufs=1,
                                name="attnb")
                for p in range(HL // 2):
                    h0, h1 = 2 * p, 2 * p + 1
                    kv = h0 // (HL // KVL)
                    qpair = qtb[:, h0 * TC:(h0 + 2) * TC]
                    pa0 = ps.tile([128, TC], F32, tag="pa", bufs=2, name="pa0")
                    pa1 = ps.tile([128, TC], F32, tag="pa", bufs=2, name="pa1")
                    pd0 = ps.tile([128, TC], F32, tag="pj", bufs=2, name="pd0")
                    pd1 = ps.tile([128, TC], F32, tag="pj", bufs=2, name="pd1")

                    def emit_pa_pd(kt, pt):
                        st, sp = (kt == 0), (kt == nkt - 1)
                        vt = vb[:, kt * KVCOLS + kv * 128:
                                kt * KVCOLS + (kv + 1) * 128]
                        nc.tensor.matmul(pa0[:], vt, pt[:, :TC],
                                         start=st, stop=sp)
                        nc.tensor.matmul(pa1[:], vt, pt[:, TC:],
                                         start=st, stop=sp)
                        nc.tensor.matmul(pd0[:], ones_sb[:], pt[:, :TC],
                                         start=st, stop=sp)
                        nc.tensor.matmul(pd1[:], ones_sb[:], pt[:, TC:],
                                         start=st, stop=sp)

                    pending = []
                    for kt in range(nkt):
                        kts = ktb[:, kv * S + kt * 128:kv * S + (kt + 1) * 128]
                        sps = ps.tile([128, 2 * TC], F32, tag="s", bufs=2,
                                      name="sps")
                        # matmul out must stay within one PSUM bank: write
                        # the fused tile's halves with two 512-wide matmuls
                        # (same stationary K tile), then ONE fused exp.
                        nc.tensor.matmul(sps[:, :TC], kts,
                                         qpair[:, :TC], start=True, stop=True)
                        nc.tensor.matmul(sps[:, TC:], kts,
                                         qpair[:, TC:], start=True, stop=True)
                        # pa/pd run TWO k-tiles behind the scores: the extra
                        # slack hides exp latency AND the previous pair's
                        # normalize chain (which frees the pa psum slots).
                        if len(pending) >= 2:
                            emit_pa_pd(*pending.pop(0))
                        pt = sb.tile([128, 2 * TC], BF16, tag="pt", bufs=4,
                                     name="pt")
                        nc.scalar.activation(pt[:], sps[:], EXP, scale=SCALE)
                        j = kt - 4 * c
                        if j >= 0:
                            # always DVE: GpSimd must stay free of compute so
                            # a collective's completion wait blocks nothing
                            msk = mask_sb[:, j * 2 * TC:(j + 1) * 2 * TC]
                            nc.vector.tensor_tensor(pt[:], pt[:], msk, MUL)
                        pending.append((kt, pt))
                    while pending:
                        emit_pa_pd(*pending.pop(0))
                    rc0 = sb.tile([128, TC], F32, tag="rc", bufs=2, name="rc0")
                    nc.vector.reciprocal_approx_fast(rc0[:], pd0[:])
                    nc.vector.tensor_tensor(
                        attnb[:, h0 * TC:(h0 + 1) * TC], pa0[:], rc0[:], MUL)
                    rc1 = sb.tile([128, TC], F32, tag="rc", bufs=2, name="rc1")
                    nc.vector.reciprocal_approx_fast(rc1[:], pd1[:])
                    nc.vector.tensor_tensor(
                        attnb[:, h1 * TC:(h1 + 1) * TC], pa1[:], rc1[:], MUL)
                    if p == 2:
                        # hoist first o-proj weight loads under attention
                        prefetch_wor(0)

            def outproj(c):
                """Local partial o_proj -> cci[rank, t, oc], then bf16
                ReduceScatter(add) within the 4-core group. The cco->out_t
                DMAs are deferred to the end of the program so no in-order
                DMA queue mid-stream carries a descriptor that has to wait
                for a collective. The last chunk's RS is split in two
                halves (even / odd output groups) so the first RS overlaps
                the second half of the out-projection matmuls."""
                t0 = c * TC
                ccis = [dr.tile([TP, TC, OC // 2], BF16, tag="cci",
                                bufs=4, name=f"cci{h}") for h in range(2)]
                order = [0, 2, 4, 6, 1, 3, 5, 7]

                pending_out = []

                def emit_rs(half):
                    """Trigger the RS for one half of the output columns.
                    The cco->out DMA rides the GpSimd stream (its wait for
                    the RS blocks only GpSimd, which hosts no other work —
                    a sync-engine DMA here would stall ALL DMA dispatch for
                    the collective's latency), and is DEFERRED past any
                    later RS trigger so back-to-back collectives don't
                    serialize on it."""
                    cco = dr.tile([TC, OC // 2], BF16, tag="cco", bufs=4,
                                  name=f"cco{half}")
                    dst = out_d[t0:t0 + TC,
                                half * (OC // 2):(half + 1) * (OC // 2)]
                    nc.gpsimd.collective_compute(
                        "ReduceScatter", ADD,
                        replica_groups=GROUPS, ins=[ccis[half][:]],
                        outs=[cco[:]])
                    pending_out.append((dst, cco))

                for i, ocg in enumerate(order):
                    wor = _WOR_PRE.pop(ocg, None)
                    if wor is None:
                        wor = prefetch_wor(ocg)
                    if i + 1 < 8:
                        prefetch_wor(order[i + 1])
                    cci = ccis[ocg % 2]
                    ot = sb.tile([128, 4 * TC], BF16, tag="ot", bufs=2,
                                 name="ot")
                    for tt in range(TC // 128):
                        po = ps.tile([128, TC], F32, tag="s", bufs=2, name="po")
                        for ft in range(HL):
                            nc.tensor.matmul(
                                po[:],
                                attnb[:, ft * TC + tt * 128:ft * TC + (tt + 1) * 128],
                                wor[:, ft * TC:(ft + 1) * TC],
                                start=(ft == 0), stop=(ft == HL - 1))
                        nc.scalar.copy(ot[:, tt * TC:(tt + 1) * TC], po[:])
                    nc.sync.dma_start(
                        cci[ocg // 2].rearrange("(tt p) oc -> p tt oc", p=128),
                        ot.rearrange("p (tt oc) -> p tt oc", oc=TC))
                    if i == 3:
                        emit_rs(0)
                emit_rs(1)
                for dst, cco in pending_out:
                    nc.gpsimd.dma_start(dst, cco[:])

            # ---- chunk loop ----
            load_xt(0)
            for c in range(NCHUNK):
                proj(c)
                attention(c)
                outproj(c)

    nc.compile()
    return nc


def _get_nc():
    if "nc" not in _BUILT:
        _BUILT["nc"] = _build()
    return _BUILT["nc"]


def kernel(hidden_states, cos, sin, wq, wk, wv, wo):
    global LAST_RESULT
    nc = _get_nc()

    hidden_states = np.asarray(hidden_states, dtype=np.float32)
    cos = np.asarray(cos, dtype=np.float32)
    sin = np.asarray(sin, dtype=np.float32)
    wq = np.asarray(wq, dtype=np.float32)
    wk = np.asarray(wk, dtype=np.float32)
    wv = np.asarray(wv, dtype=np.float32)
    wo = np.asarray(wo, dtype=np.float32)

    # host-side shard prep (bf16, panel-major so every DMA is contiguous)
    xts = [np.ascontiguousarray(hidden_states[b].T).astype(B16NP)
           for b in range(B)]
    cts = [np.ascontiguousarray(cos[b].T).astype(B16NP) for b in range(B)]
    sin_eff = []
    for b in range(B):
        se = np.ascontiguousarray(sin[b].T).copy()
        se[0:64, :] *= -1.0
        sin_eff.append(se.astype(B16NP))

    # fused pair masks: each diagonal mask duplicated for the head pair
    masks = np.zeros((HD, 8 * TC), dtype=B16NP)
    for j in range(4):
        m = ((np.arange(HD)[:, None] + 128 * j)
             <= np.arange(TC)[None, :]).astype(B16NP)
        masks[:, j * 2 * TC:j * 2 * TC + TC] = m
        masks[:, j * 2 * TC + TC:(j + 1) * 2 * TC] = m
    onesb = np.ones((128, 128), dtype=B16NP)

    in_maps = []
    for core in range(N_CORES):
        b, r = core // TP, core % TP
        wq_s = wq[:, r * QCOLS:(r + 1) * QCOLS]
        wk_s = wk[:, r * KVCOLS:(r + 1) * KVCOLS]
        wv_s = wv[:, r * KVCOLS:(r + 1) * KVCOLS]
        wo_s = wo[r * QCOLS:(r + 1) * QCOLS, :]
        wqp = np.ascontiguousarray(
            wq_s.reshape(KT, 128, HL, 128).transpose(2, 1, 0, 3)
            .reshape(HL, 128, KT * 128)).astype(B16NP)
        wkp = np.ascontiguousarray(
            wk_s.reshape(KT, 128, KVL, 128).transpose(2, 1, 0, 3)
            .reshape(KVL, 128, KT * 128)).astype(B16NP)
        wvp = np.ascontiguousarray(
            wv_s.reshape(KT, 128, KVCOLS).transpose(1, 0, 2)
            .reshape(128, KT * KVCOLS)).astype(B16NP)
        wop = np.ascontiguousarray(
            wo_s.reshape(HL, 128, H)).astype(B16NP)
        in_maps.append({
            "xt": xts[b],
            "cos_t": cts[b],
            "sin_t": sin_eff[b],
            "wqp": wqp,
            "wkp": wkp,
            "wvp": wvp,
            "wop": wop,
            "masks": masks,
            "onesb": onesb,
        })

    res = run_bass_kernel_spmd(nc, in_maps, core_ids=list(range(N_CORES)))
    LAST_RESULT = res

    out = np.empty((B, S, H), dtype=np.float32)
    for core in range(N_CORES):
        b, r = core // TP, core % TP
        out[b, :, r * OC:(r + 1) * OC] = res.results[core]["out_t"].astype(
            np.float32)
    return out
